# revision 1
# baseline (speedup 1.0000x reference)
"""Causal self-attention (B=2, T=2048, D=768, H=12) on 8 TRN2 NeuronCores.

Sharding: tensor-parallel over (batch, head) pairs. 24 pairs / 8 cores = 3
heads per core, all from one batch. Each core computes q/k in transposed
[head_dim, T] layout straight out of the QKV projection, runs causal
attention per head (scores^T = K^T.T-style matmuls, exp on ScalarE, softmax
denominator via a fused ones-column in the PV matmul), then a partial output
projection over its 3 heads' rows of w_out. The host sums the 4 partial
outputs per batch and adds b_out.
"""

import numpy as np
import ml_dtypes

import concourse.bass as bass
import concourse.bacc as bacc
import concourse.mybir as mybir
import concourse.tile as tile
from concourse.masks import make_upper_triangular
from concourse.bass_utils import run_bass_kernel_spmd

B, T, D, H, HD = 2, 2048, 768, 12, 64
NCORES = 8
HPC = 3            # heads per core
CPB = NCORES // B  # cores per batch = 4
CC = D // 128      # d_model chunks of 128 = 6
CCK = CC + 1       # contraction chunks incl. bias ones-row chunk
TW = T // 512      # token windows of 512 = 4
KC = T // 128      # k chunks of 128 = 16
SCALE = 1.0 / float(np.sqrt(HD))

BF = mybir.dt.bfloat16
F32 = mybir.dt.float32
NBF = ml_dtypes.bfloat16

EXP_BATCH = 2  # score chunks per exp call (PSUM tile = 2 banks)


def _attn_qw(nc, streams, qw, pools):
    """Emit attention for one q-window for a list of head streams.

    streams: list of dicts with keys:
      qq, kk : SBUF [128, T] tiles holding q^T/k^T (two 64-row halves)
      rows   : list of (row_base, kc_parity) "lanes"; for a 2-head pair the
               two streams each use one half; for the dup-packed single head
               both halves hold the same head so lanes alternate k-chunks.
      h      : head index (0..2) within this core
      vt     : vT3 tile
      yn     : yn tile
    """
    poolS, poolPS, poolE, poolSC, poolRB = (
        pools["S"], pools["PS"], pools["E"], pools["SC"], pools["RB"])
    mask_tri = pools["mask"]
    nchunks = 4 * (qw + 1)
    qs = qw * 512

    for st in streams:
        st["y"] = poolPS.tile([HD + 1, 512], F32, tag="ypv",
                              name=f"y_h{st['h']}_q{qw}")

    # batches of up to EXP_BATCH k-chunks, per stream. Diagonal (masked)
    # chunks go FIRST (mask chain off the window-tail critical path), in
    # ascending j so the start=True PV matmul (j=0) writes the full column
    # range before trimmed chunks accumulate sub-ranges of it.
    kc_order = list(range(4 * qw, nchunks)) + list(range(4 * qw))[::-1]
    for b0 in range(0, nchunks, EXP_BATCH):
        kcs = kc_order[b0:b0 + EXP_BATCH]
        nb = len(kcs)
        for st in streams:
            h = st["h"]
            s_ps = poolS.tile([128, EXP_BATCH, 512], F32, tag="s3",
                              name=f"s_h{h}_q{qw}_k{b0}")
            et = poolE.tile([128, EXP_BATCH, 512], BF, tag="et",
                            name=f"e_h{h}_q{qw}_k{b0}")
            # columns < 128*jmin are fully masked for every chunk in this
            # batch: skip them in both the matmuls and the exp
            jmin = min(max(0, kc - 4 * qw) for kc in kcs)
            for i in range(nb):
                kc = kcs[i]
                rb_, _ = st["rows"][kc % len(st["rows"])]
                nc.tensor.matmul(
                    s_ps[:, i, 128 * jmin:512],
                    lhsT=st["kk"][rb_:rb_ + HD, kc * 128:(kc + 1) * 128],
                    rhs=st["qq"][rb_:rb_ + HD, qs + 128 * jmin:qs + 512],
                    start=True, stop=True,
                )
            # exp over the whole batch (ScalarE), PSUM -> SBUF bf16
            nc.scalar.activation(
                out=et[:, 0:nb, 128 * jmin:512],
                in_=s_ps[:, 0:nb, 128 * jmin:512],
                func=mybir.ActivationFunctionType.Exp, scale=SCALE,
            )
            # causal masking on diagonal chunks (transition block only)
            for i in range(nb):
                kc = kcs[i]
                j = kc - 4 * qw
                if j < 0:
                    continue  # fully below diagonal: keep all
                nc.gpsimd.tensor_mul(
                    out=et[:, i, 128 * j:128 * (j + 1)],
                    in0=et[:, i, 128 * j:128 * (j + 1)],
                    in1=mask_tri,
                )
            # PV accumulation: lhsT = [V | ones] (65 cols), rhs = E^T.
            # Diagonal chunks contribute nothing below column 128*j, so the
            # rhs is trimmed; the j=0 chunk ran first with start=True and
            # wrote the full range, so sub-range accumulation is safe.
            for i in range(nb):
                kc = kcs[i]
                j = max(0, kc - 4 * qw)
                idx = b0 + i
                nc.tensor.matmul(
                    st["y"][:, 128 * j:512],
                    lhsT=st["vt"][:, kc, st["h"], 0:HD + 1],
                    rhs=et[:, i, 128 * j:512],
                    start=(idx == 0), stop=(idx == nchunks - 1),
                    skip_group_check=True,
                )

    # normalize: yn = y[0:64] / sumexp (row 64), cast to bf16.
    # st["yn_ap"](qs) gives the destination slice (may be a shifted
    # partition range -- DVE supports differing src/dst base partitions).
    F32R = mybir.dt.float32r
    for st in streams:
        h = st["h"]
        y = st["y"]
        sc = poolSC.tile([128, 512], F32R, tag="sc", name=f"sc_h{h}_q{qw}")
        rb = poolRB.tile([HD, 512], F32, tag="rb", name=f"rb_h{h}_q{qw}")
        # reciprocal of sumexp, kept on partition 64 (aligned with source),
        # written as fp32r so the broadcast matmul can consume it
        with nc.allow_low_precision(reason="fp32r == fp32 bits; rounding "
                                    "only affects the PE broadcast matmul"):
            nc.vector.reciprocal(out=sc[HD:HD + 1, :], in_=y[HD:HD + 1, :])
        # broadcast to 64 partitions with a K=1 fp32r matmul against a ones
        # row (full-rate for N>=256); then evacuate to SBUF for the multiply
        rbps = poolS.tile([HD, 512], F32, tag="s3", name=f"rbps_h{h}_q{qw}")
        nc.tensor.matmul(
            rbps,
            lhsT=pools["ones"][HD:HD + 1, 0:HD],
            rhs=sc[HD:HD + 1, :],
            start=True, stop=True,
        )
        nc.any.tensor_copy(out=rb, in_=rbps)
        nc.vector.tensor_mul(
            out=st["yn_ap"](qs),
            in0=y[0:HD, :], in1=rb[:, :],
        )


def build_bass():
    nc = bacc.Bacc(None, target_bir_lowering=False)

    xT = nc.dram_tensor("xT", [CC, 128, T], BF, kind="ExternalInput")
    wqk = nc.dram_tensor("wqk", [CCK, 128, 3, 128], BF, kind="ExternalInput")
    wv = nc.dram_tensor("wv", [CC, 128, HPC * HD], BF, kind="ExternalInput")
    wo = nc.dram_tensor("wo", [HPC, HD, D], BF, kind="ExternalInput")
    outT = nc.dram_tensor("outT", [D, T], F32, kind="ExternalOutput")

    with tile.TileContext(nc) as tc:
        with (
            tc.tile_pool(name="big", bufs=1) as big,
            tc.tile_pool(name="ets", bufs=6) as ets,
            tc.tile_pool(name="scr", bufs=4) as scr,
            tc.tile_pool(name="outs", bufs=6) as outs,
            tc.tile_pool(name="psS", bufs=2, space="PSUM") as poolS,
            tc.tile_pool(name="psA", bufs=2, space="PSUM") as poolPS,
        ):
            # ---- constants / inputs in SBUF ----
            # weights first (small), then x in token-window-major order so
            # the first token window's projection completes after ~1MB of
            # traffic instead of the full 3.7MB
            wqks = big.tile([128, CCK, 3, 128], BF, tag="wqk")
            wvs = big.tile([128, CC, HPC * HD], BF, tag="wv")
            xTs = big.tile([128, CCK, T], BF, tag="xT")
            # bias chunk: only partition 0 (the ones row) is ever read --
            # the bias matmul below uses K=1 -- so no DMA and no zero-fill
            nc.gpsimd.memset(xTs[0:1, CC, :], 1.0)
            nc.sync.dma_start(
                out=wqks[:, 0:4], in_=wqk[0:4].rearrange("c p a f -> p c a f"))
            for cc in range(4):
                nc.sync.dma_start(out=xTs[:, cc, 0:512], in_=xT[cc, :, 0:512])
            nc.sync.dma_start(
                out=wqks[:, 4:CCK],
                in_=wqk[4:CCK].rearrange("c p a f -> p c a f"))
            for cc in range(4, CC):
                nc.sync.dma_start(out=xTs[:, cc, 0:512], in_=xT[cc, :, 0:512])
            nc.sync.dma_start(out=wvs, in_=wv.rearrange("c p f -> p c f"))
            for tw in range(1, TW):
                for cc in range(CC):
                    nc.sync.dma_start(
                        out=xTs[:, cc, tw * 512:(tw + 1) * 512],
                        in_=xT[cc, :, tw * 512:(tw + 1) * 512])
            # w_out rows: heads 0+1 stacked to 128 partitions, head 2 alone
            wos01 = big.tile([128, D], BF, tag="wo01")
            nc.sync.dma_start(out=wos01,
                              in_=wo[0:2].rearrange("h p e -> (h p) e"))
            wos2 = big.tile([HD, D], BF, tag="wo2")
            nc.sync.dma_start(out=wos2, in_=wo[2])

            mask_tri = big.tile([128, 128], BF, tag="mask")
            make_upper_triangular(nc, mask_tri, val=1.0, diag=True)
            ones_stage = big.tile([128, HD], F32, tag="ones_stage")
            nc.vector.memset(ones_stage, 1.0)
            ones_t = big.tile([128, HD], mybir.dt.float32r, tag="ones")
            with nc.allow_low_precision(reason="fp32r ones for normalizer "
                                        "broadcast matmul"):
                nc.vector.tensor_copy(out=ones_t, in_=ones_stage)

            # q^T/k^T feature-chunk tiles: QQ=[h0q|h1q], KK=[h0k|h1k],
            # QQ2=[h2q|h2q], KK2=[h2k|h2k]
            qk_tiles = []
            for nm in ("QQ", "KK", "QQ2", "KK2"):
                t_ = big.tile([128, T], BF, tag=nm, name=nm)
                qk_tiles.append(t_)

            # token-major V (+ ones column), per head: [128, kc, h, 66]
            vT3 = big.tile([128, KC, HPC, 66], BF, tag="vT3")
            for h in range(HPC):
                nc.gpsimd.memset(vT3[:, :, h, HD:HD + 1], 1.0)

            # normalized attention outputs: heads 0+1 stacked on 128
            # partitions, head 2 on its own 64-partition tile
            ynA = big.tile([128, T], BF, tag="ynA")
            ynB = big.tile([HD, T], BF, tag="ynB")

            QQ, KK, QQ2, KK2 = qk_tiles
            pools = {"S": poolS, "PS": poolPS, "E": ets, "SC": scr,
                     "RB": scr, "mask": mask_tri,
                     "ones": ones_t}

            # ---- interleaved: per token-window, project then attend ----
            for tw in range(TW):
                ts_ = tw * 512
                # q^T/k^T projection for this token window.
                # fc0=[h0q|h1q], fc1=[h0k|h1k], fc2=[h2q|h2k]; fc2's halves
                # are fanned out (duplicated) into QQ2/KK2 via DVE copies so
                # h2 scores can row-pack two k-chunks.
                for fc in range(3):
                    ps = poolPS.tile([128, 512], F32, tag="acc",
                                     name=f"ps_f{fc}_t{tw}")
                    for cc in range(CC):
                        nc.tensor.matmul(
                            ps,
                            lhsT=wqks[:, cc, fc, :],
                            rhs=xTs[:, cc, ts_:ts_ + 512],
                            start=(cc == 0), stop=False,
                        )
                    nc.tensor.matmul(
                        ps,
                        lhsT=wqks[0:1, CC, fc, :],
                        rhs=xTs[0:1, CC, ts_:ts_ + 512],
                        start=False, stop=True,
                    )
                    if fc < 2:
                        nc.any.tensor_copy(
                            out=qk_tiles[fc][:, ts_:ts_ + 512], in_=ps,
                        )
                    else:
                        for dst_half in (0, HD):
                            nc.any.tensor_copy(
                                out=QQ2[dst_half:dst_half + HD, ts_:ts_ + 512],
                                in_=ps[0:HD, :],
                            )
                            nc.any.tensor_copy(
                                out=KK2[dst_half:dst_half + HD, ts_:ts_ + 512],
                                in_=ps[HD:128, :],
                            )
                # token-major V projection for this window's 4 k-chunks
                for tc_i in range(4 * tw, 4 * tw + 4):
                    psv = poolPS.tile([128, 512], F32, tag="acc",
                                      name=f"psv_{tc_i}")
                    for cc in range(CC):
                        nc.tensor.matmul(
                            psv[:, 0:HPC * HD],
                            lhsT=xTs[:, cc, tc_i * 128:(tc_i + 1) * 128],
                            rhs=wvs[:, cc, :],
                            start=(cc == 0), stop=(cc == CC - 1),
                        )
                    nc.any.tensor_copy(
                        out=vT3[:, tc_i, :, 0:HD],
                        in_=psv[:, 0:HPC * HD].rearrange(
                            "p (h d) -> p h d", h=HPC),
                    )

                # attention for q-window tw (all needed k-chunks are ready)
                qw = tw
                qs = qw * 512
                pair = [
                    {"qq": QQ, "kk": KK, "rows": [(0, 0)], "h": 0, "vt": vT3,
                     "yn_ap": lambda q: ynA[0:HD, q:q + 512]},
                    {"qq": QQ, "kk": KK, "rows": [(HD, 0)], "h": 1, "vt": vT3,
                     "yn_ap": lambda q: ynA[HD:128, q:q + 512]},
                ]
                _attn_qw(nc, pair, qw, pools)
                solo = [
                    {"qq": QQ2, "kk": KK2, "rows": [(0, 0), (HD, 0)], "h": 2,
                     "vt": vT3, "yn_ap": lambda q: ynB[0:HD, q:q + 512]},
                ]
                _attn_qw(nc, solo, qw, pools)
                # previous window's output projection sits here in the PE
                # stream: its gate (that window's normalize chain) is long
                # done, and it fills PE while this window's solo normalize
                # chain drains
                if qw >= 1:
                    _outproj(nc, qw - 1, wos01, wos2, ynA, ynB, poolPS, outs,
                             outT)

            _outproj(nc, TW - 1, wos01, wos2, ynA, ynB, poolPS, outs, outT)
    return nc


def _outproj(nc, qw, wos01, wos2, ynA, ynB, poolPS, outs, outT):
    qs = qw * 512
    for ec in range(CC):
        ops = poolPS.tile([128, 512], F32, tag="ypv",
                          name=f"ops_e{ec}_q{qw}")
        nc.tensor.matmul(
            ops,
            lhsT=wos01[:, ec * 128:(ec + 1) * 128],
            rhs=ynA[:, qs:qs + 512],
            start=True, stop=False,
        )
        nc.tensor.matmul(
            ops,
            lhsT=wos2[:, ec * 128:(ec + 1) * 128],
            rhs=ynB[:, qs:qs + 512],
            start=False, stop=True,
        )
        osb = outs.tile([128, 512], F32, tag="osb", name=f"osb_e{ec}_q{qw}")
        nc.any.tensor_copy(out=osb, in_=ops)
        nc.sync.dma_start(
            out=outT[ec * 128:(ec + 1) * 128, qs:qs + 512],
            in_=osb,
        )


def _prep_core_inputs(c, x, w_qkv, b_qkv, w_out):
    b = c // CPB
    g = c % CPB
    hs = [HPC * g + i for i in range(HPC)]

    qc = [np.arange(h * HD, (h + 1) * HD) for h in hs]
    kc_ = [D + h * HD + np.arange(HD) for h in hs]
    vc = [2 * D + h * HD + np.arange(HD) for h in hs]

    cols = np.concatenate([qc[0], qc[1], kc_[0], kc_[1], qc[2], kc_[2]])
    vcols = np.concatenate(vc)

    xT = np.ascontiguousarray(x[b].T).astype(np.float32)
    # bias row for the K=1 bias matmul lives in wqk chunk CC, row 0
    wqk = np.zeros((CCK * 128, 384), dtype=np.float32)
    wqk[0:D] = w_qkv[:, cols]
    wqk[D] = b_qkv[cols]
    wv = w_qkv[:, vcols].astype(np.float32)
    wo = np.stack([w_out[h * HD:(h + 1) * HD, :] for h in hs]).astype(NBF)

    return {
        "xT": np.ascontiguousarray(xT.astype(NBF).reshape(CC, 128, T)),
        "wqk": np.ascontiguousarray(wqk.astype(NBF).reshape(CCK, 128, 3, 128)),
        "wv": np.ascontiguousarray(wv.astype(NBF).reshape(CC, 128, HPC * HD)),
        "wo": np.ascontiguousarray(wo),
    }


_NC_CACHE = {}


def get_nc():
    if "nc" not in _NC_CACHE:
        nc = build_bass()
        nc.finalize()  # Bacc: run reg-alloc + sync-wait splitting passes
        _NC_CACHE["nc"] = nc
    return _NC_CACHE["nc"]


def kernel(x, w_qkv, b_qkv, w_out, b_out, _run_kwargs=None):
    x = np.asarray(x, dtype=np.float32)
    w_qkv = np.asarray(w_qkv, dtype=np.float32)
    b_qkv = np.asarray(b_qkv, dtype=np.float32)
    w_out = np.asarray(w_out, dtype=np.float32)
    b_out = np.asarray(b_out, dtype=np.float32)

    nc = get_nc()
    in_maps = [_prep_core_inputs(c, x, w_qkv, b_qkv, w_out)
               for c in range(NCORES)]
    kwargs = dict(_run_kwargs or {})
    res = run_bass_kernel_spmd(nc, in_maps, core_ids=list(range(NCORES)),
                               **kwargs)
    if kwargs:
        _NC_CACHE["last_results"] = res

    bv_corr = b_qkv[2 * D:3 * D] @ w_out  # [D]
    out = np.zeros((B, T, D), dtype=np.float32)
    for b in range(B):
        acc = np.zeros((T, D), dtype=np.float32)
        for g in range(CPB):
            acc += np.asarray(res.results[b * CPB + g]["outT"]).T
        out[b] = acc + (b_out + bv_corr)[None, :]
    return out


if __name__ == "__main__":
    # smoke build
    nc = build_bass()
    print("built OK; instructions:",
          sum(1 for _ in nc.m.functions[0].instructions)
          if hasattr(nc.m.functions[0], "instructions") else "?")



# revision 6
# speedup vs baseline: 1.0697x; 1.0697x over previous
"""Causal self-attention (B=2, T=2048, D=768, H=12) on 8 TRN2 NeuronCores.

Sharding: tensor-parallel over (batch, head) pairs; 3 heads per core, one
batch per 4-core group. All on-device tensors are fp16 (same PE/DVE cost as
bf16 under the cost model, 8x less rounding noise). Per 512-token window:
q/k/v projection, then causal attention with the scores->exp->PV chain
software-pipelined in "rounds" of 2-k-chunk batches across the 3 head
streams, with next-window projection and previous-window output-projection
matmul groups interleaved between rounds to keep the PE busy while ScalarE
exp latency drains. QKV bias is folded into the PSUM->SBUF evacuation
(DVE tensor_scalar). Host sums the 4 partial outputs per batch and adds
b_out (+ the v-bias correction through w_out).
"""

import numpy as np

import concourse.bass as bass
import concourse.bacc as bacc
import concourse.mybir as mybir
import concourse.tile as tile
from concourse import library_config
from concourse.masks import make_upper_triangular
from concourse.bass_utils import run_bass_kernel_spmd

B, T, D, H, HD = 2, 2048, 768, 12, 64
NCORES = 8
HPC = 3            # heads per core
CPB = NCORES // B  # cores per batch = 4
CC = D // 128      # d_model chunks of 128 = 6
TW = T // 512      # token windows of 512 = 4
KC = T // 128      # k chunks of 128 = 16
SCALE = 1.0 / float(np.sqrt(HD))

F16 = mybir.dt.float16
F32 = mybir.dt.float32
F32R = mybir.dt.float32r

EXP_BATCH = 2  # k-chunks per exp call / per s-tile (PSUM tile = 2 banks)

# Schraudolph fast-exp on DVE/Pool for stream h2's below-diagonal batches in
# late windows (relieves the ScalarE bottleneck there). exp(x) ~ bf16 bitcast
# of int16(A*x + B); ~2.4% RMS approximation error on ~11% of the attention
# weights => ~8e-3 end-to-end rel err (budget 2e-2).
SCHRAU = False
SCHRAU_A = 128.0 / np.log(2.0)
SCHRAU_B = float(127 << 7) - 7.5


def build_bass():
    nc = bacc.Bacc(None, target_bir_lowering=False)

    xT = nc.dram_tensor("xT", [CC, 128, T], F16, kind="ExternalInput")
    wqk = nc.dram_tensor("wqk", [CC, 128, 3, 128], F16, kind="ExternalInput")
    bqk = nc.dram_tensor("bqk", [128, 3], F32, kind="ExternalInput")
    wv = nc.dram_tensor("wv", [CC, 128, HPC * HD], F16, kind="ExternalInput")
    wo01d = nc.dram_tensor("wo01", [128, D], F16, kind="ExternalInput")
    wo2d = nc.dram_tensor("wo2", [HD, D], F16, kind="ExternalInput")
    outT = nc.dram_tensor("outT", [D, T], F16, kind="ExternalOutput")

    with tile.TileContext(nc) as tc:
        with (
            tc.tile_pool(name="big", bufs=1) as big,
            tc.tile_pool(name="ets", bufs=6) as ets,
            tc.tile_pool(name="scr", bufs=3) as scr,
            tc.tile_pool(name="outs", bufs=4) as outs,
            tc.tile_pool(name="psS", bufs=2, space="PSUM") as psS,
            tc.tile_pool(name="psY", bufs=3, space="PSUM") as psY,
            tc.tile_pool(name="psA", bufs=1, space="PSUM") as psA,
        ):
            # ---- SBUF persistent tiles ----
            wqks = big.tile([128, CC, 3, 128], F16, tag="wqk")
            wvs = big.tile([128, CC, HPC * HD], F16, tag="wv")
            xTs = big.tile([128, CC, T], F16, tag="xT")
            bqks = big.tile([128, 3], F32, tag="bqk")
            wos01 = big.tile([128, D], F16, tag="wo01")
            wos2 = big.tile([HD, D], F16, tag="wo2")
            QQ = big.tile([128, T], F16, tag="QQ")
            KK = big.tile([128, T], F16, tag="KK")
            QQ2 = big.tile([HD, T], F16, tag="QQ2")
            KK2 = big.tile([HD, T], F16, tag="KK2")
            # token-major V (+ ones column at 64): [128, kc, h, 66]
            vT3 = big.tile([128, KC, HPC, 66], F16, tag="vT3")
            ynA = big.tile([128, T], F16, tag="ynA")
            ynB = big.tile([HD, T], F16, tag="ynB")
            mask_tri = big.tile([128, 128], F16, tag="mask")
            ones_t = big.tile([128, HD], F32R, tag="ones")

            # load the GPSIMD ucode library that carries partition_broadcast
            nc.gpsimd.load_library(library_config.proxy)

            # PE p-state warm-up: a dense run of ~free N=1 matmuls starts
            # the tensor engine's ramp clock during the input-DMA wait so
            # the real matmuls reach full clock ~2us earlier; the dummy Exp
            # pulls the activation-table load (1.3us) off the first real
            # exp's critical path.
            wtiny = big.tile([1, 8], F16, tag="wtiny")
            nc.vector.memset(wtiny, 0.5)
            nc.scalar.activation(out=wtiny[0:1, 4:8], in_=wtiny[0:1, 0:4],
                                 func=mybir.ActivationFunctionType.Exp)
            s_warm = psS.tile([128, EXP_BATCH, 512], F32, tag="s3",
                              name="s_warm")
            for i in range(550):
                nc.tensor.matmul(
                    s_warm[0:1, i // 512, i % 512:i % 512 + 1],
                    lhsT=wtiny[0:1, 0:1], rhs=wtiny[0:1, 0:1],
                    start=True, stop=True, skip_group_check=True)

            # ---- input DMAs: window-0 criticals first. wqk goes through
            # the SP HWDGE queue while xT window-0 chunks go through the
            # Pool SWDGE queue -- two parallel descriptor-generation paths.
            nc.gpsimd.dma_start(out=xTs[:, 0, 0:512], in_=xT[0, :, 0:512])
            nc.gpsimd.dma_start(out=xTs[:, 3, 0:512], in_=xT[3, :, 0:512])
            nc.gpsimd.dma_start(out=xTs[:, 4, 0:512], in_=xT[4, :, 0:512])
            nc.gpsimd.dma_start(out=xTs[:, 5, 0:512], in_=xT[5, :, 0:512])
            nc.sync.dma_start(out=wqks[:, 0, 0:1], in_=wqk[0, :, 0:1])
            nc.sync.dma_start(out=wqks[:, 0, 1:3], in_=wqk[0, :, 1:3])
            nc.sync.dma_start(out=bqks, in_=bqk[:, :])
            nc.sync.dma_start(out=xTs[:, 1, 0:512], in_=xT[1, :, 0:512])
            nc.sync.dma_start(out=wqks[:, 1], in_=wqk[1])
            nc.sync.dma_start(out=xTs[:, 2, 0:512], in_=xT[2, :, 0:512])
            for cc in range(2, CC):
                nc.sync.dma_start(out=wqks[:, cc], in_=wqk[cc])
                if cc == 2:
                    nc.sync.dma_start(out=wvs,
                                      in_=wv.rearrange("c p f -> p c f"))
            nc.sync.dma_start(out=wos01, in_=wo01d[:, :])
            nc.sync.dma_start(out=wos2, in_=wo2d[:, :])
            for cc in range(CC):
                nc.sync.dma_start(out=xTs[:, cc, 512:T],
                                  in_=xT[cc, :, 512:T])

            # ---- constants ----
            make_upper_triangular(nc, mask_tri, val=1.0, diag=True)
            ones_stage = big.tile([128, HD], F32, tag="ones_stage")
            nc.vector.memset(ones_stage, 1.0)
            with nc.allow_low_precision(reason="fp32r ones for normalizer "
                                        "broadcast matmul"):
                nc.vector.tensor_copy(out=ones_t, in_=ones_stage)
            for h in range(HPC):
                nc.gpsimd.memset(vT3[:, :, h, HD:HD + 1], 1.0)

            st = {
                "pending_norm": [],
                "wqks": wqks, "wvs": wvs, "xTs": xTs, "bqks": bqks,
                "wos01": wos01, "wos2": wos2, "QQ": QQ, "KK": KK,
                "QQ2": QQ2, "KK2": KK2, "vT3": vT3, "ynA": ynA, "ynB": ynB,
                "mask": mask_tri, "ones": ones_t,
                "psS": psS, "psY": psY, "psA": psA,
                "ets": ets, "scr": scr, "outs": outs, "outT": outT,
            }

            # prologue: window-0 q/k projection, cc-major across three
            # psY accumulators so each arriving x chunk is consumed
            # immediately; V chunks 0/1 here, 2/3 ride window 0's bg queue.
            paccs = [psY.tile([128, 512], F32, tag="y", name=f"pacc_f{fc}")
                     for fc in range(3)]
            for cc in range(CC):
                for fc in range(3):
                    nc.tensor.matmul(
                        paccs[fc],
                        lhsT=wqks[:, cc, fc, :],
                        rhs=xTs[:, cc, 0:512],
                        start=(cc == 0), stop=(cc == CC - 1),
                    )
            for fc in range(3):
                _evac_qk(nc, st, 0, fc, paccs[fc])
            _proj_v_chunk(nc, st, 0)
            _proj_v_chunk(nc, st, 1, acc="y")

            for w in range(TW):
                _attn_window(nc, st, w)

            # epilogue: output projection for the last window, split-phase:
            # the h0/h1 contraction runs over 6 parallel accumulators
            # (borrowing the now-idle psS/psY banks) while the h2 stream's
            # normalize drains, then the h2 matmuls close each group.
            qs = (TW - 1) * 512
            opst = []
            for ec in range(CC):
                pool, tag = [(psY, "y"), (psY, "y"), (psY, "y"),
                             (psS, "s3"), (psS, "s3"), (psA, "acc")][ec]
                opst.append(pool.tile([128, 512], F32, tag=tag,
                                      name=f"opse_{ec}"))
            for c0 in (0, 256):
                for ec in range(CC):
                    nc.tensor.matmul(
                        opst[ec][:, c0:c0 + 256],
                        lhsT=wos01[:, ec * 128:(ec + 1) * 128],
                        rhs=ynA[:, qs + c0:qs + c0 + 256],
                        start=(c0 == 0), stop=False, skip_group_check=True,
                    )
            osb6e = outs.tile([128, CC, 512], F16, tag="osb",
                              name="osb_epi")
            for c0 in (0, 256):
                for ec in range(CC):
                    nc.tensor.matmul(
                        opst[ec][:, c0:c0 + 256],
                        lhsT=wos2[:, ec * 128:(ec + 1) * 128],
                        rhs=ynB[:, qs + c0:qs + c0 + 256],
                        start=False, stop=True, skip_group_check=True,
                    )
                    if c0 == 256:
                        if ec % 2 == 0:
                            nc.vector.tensor_copy(out=osb6e[:, ec, :],
                                                  in_=opst[ec])
                        else:
                            nc.scalar.copy(out=osb6e[:, ec, :], in_=opst[ec])
                        if ec % 2 == 1:
                            nc.sync.dma_start(
                                out=outT[(ec - 1) * 128:(ec + 1) * 128,
                                         qs:qs + 512].rearrange(
                                    "(e p) c -> p e c", e=2),
                                in_=osb6e[:, ec - 1:ec + 1, :],
                            )
    return nc


def _proj_qk_window(nc, st, w):
    """q/k projection for token window w: 3 fc groups of 6 matmuls each,
    bias folded into the DVE evacuation."""
    ts_ = w * 512
    for fc in range(3):
        _proj_qk_group(nc, st, w, fc)
    del ts_


def _proj_qk_group(nc, st, w, fc, acc="acc"):
    ts_ = w * 512
    pool = st["psY"] if acc == "y" else st["psA"]
    ps = pool.tile([128, 512], F32, tag=acc, name=f"ps_f{fc}_t{w}")
    for cc in range(CC):
        nc.tensor.matmul(
            ps,
            lhsT=st["wqks"][:, cc, fc, :],
            rhs=st["xTs"][:, cc, ts_:ts_ + 512],
            start=(cc == 0), stop=(cc == CC - 1),
        )
    _evac_qk(nc, st, w, fc, ps)


def _evac_qk(nc, st, w, fc, ps):
    ts_ = w * 512
    # evacuate with bias add (per-partition scalar).
    # fc0 = [h0q|h1q] -> QQ; fc1 = [h2q|h2k] -> QQ2/KK2; fc2 = [h0k|h1k] -> KK
    if fc == 0:
        nc.vector.tensor_scalar(
            out=st["QQ"][:, ts_:ts_ + 512], in0=ps,
            scalar1=st["bqks"][:, fc:fc + 1], scalar2=None,
            op0=mybir.AluOpType.add)
    elif fc == 2:
        nc.vector.tensor_scalar(
            out=st["KK"][:, ts_:ts_ + 512], in0=ps,
            scalar1=st["bqks"][:, fc:fc + 1], scalar2=None,
            op0=mybir.AluOpType.add)
    else:
        nc.vector.tensor_scalar(
            out=st["QQ2"][:, ts_:ts_ + 512], in0=ps[0:HD, :],
            scalar1=st["bqks"][0:HD, fc:fc + 1], scalar2=None,
            op0=mybir.AluOpType.add)
        nc.vector.tensor_scalar(
            out=st["KK2"][:, ts_:ts_ + 512], in0=ps[HD:128, :],
            scalar1=st["bqks"][HD:128, fc:fc + 1], scalar2=None,
            op0=mybir.AluOpType.add)


def _proj_v_chunk(nc, st, tc_i, acc="acc"):
    """token-major V projection for one 128-token chunk."""
    pool = st["psY"] if acc == "y" else st["psA"]
    psv = pool.tile([128, 512], F32, tag=acc, name=f"psv_{tc_i}")
    for cc in range(CC):
        nc.tensor.matmul(
            psv[:, 0:HPC * HD],
            lhsT=st["xTs"][:, cc, tc_i * 128:(tc_i + 1) * 128],
            rhs=st["wvs"][:, cc, :],
            start=(cc == 0), stop=(cc == CC - 1),
        )
    nc.vector.tensor_copy(
        out=st["vT3"][:, tc_i, :, 0:HD],
        in_=psv[:, 0:HPC * HD].rearrange("p (h d) -> p h d", h=HPC),
    )


def _outproj_group(nc, st, w, ec):
    qs = w * 512
    act_ok = w < TW - 2  # evacs run in window w+1; Act has slack if w+1<=2
    if ec == 0:
        st["osb6"] = st["outs"].tile([128, CC, 512], F16, tag="osb",
                                     name=f"osb_q{w}")
    ops = st["psA"].tile([128, 512], F32, tag="acc", name=f"ops_e{ec}_q{w}")
    nc.tensor.matmul(
        ops,
        lhsT=st["wos01"][:, ec * 128:(ec + 1) * 128],
        rhs=st["ynA"][:, qs:qs + 512],
        start=True, stop=False,
    )
    nc.tensor.matmul(
        ops,
        lhsT=st["wos2"][:, ec * 128:(ec + 1) * 128],
        rhs=st["ynB"][:, qs:qs + 512],
        start=False, stop=True,
    )
    nc.vector.tensor_copy(out=st["osb6"][:, ec, :], in_=ops)
    if ec == CC - 1:
        nc.sync.dma_start(
            out=st["outT"][:, qs:qs + 512].rearrange(
                "(e p) c -> p e c", e=CC),
            in_=st["osb6"],
        )


def _attn_window(nc, st, w):
    """Attention for q-window w across the 3 head streams, with background
    PE work (next-window projection, previous-window outproj) interleaved
    between score/PV rounds."""
    qs = w * 512
    nchunks = 4 * (w + 1)
    # below-diagonal chunks first (descending, so the first PV write is the
    # full column range), diagonal chunks last: the window's own K-side
    # projection (fc2) and V chunks then slide into this window's early
    # rounds instead of crowding the previous one.
    kc_order = list(range(4 * w))[::-1] + list(range(4 * w, nchunks))
    batches = [kc_order[i:i + EXP_BATCH]
               for i in range(0, nchunks, EXP_BATCH)]
    n_diag_batches = 2

    # bg_must: work that must land before this window's diagonal rounds.
    # bg_opt: deferrable work (previous window's normalize phase B and
    # outproj, next window's Q-side projection).
    bg_must = []
    if w == 0:
        for j in (2, 3):
            bg_must.append(lambda j=j: _proj_v_chunk(nc, st, j))
    else:
        bg_must.append(lambda: _proj_qk_group(nc, st, w, 2))
        for j in range(4):
            bg_must.append(lambda j=j: _proj_v_chunk(nc, st, 4 * w + j))
    bg_opt = []
    for s_, w_ in st.pop("pending_norm", []):
        bg_opt.append(lambda s_=s_, w_=w_: _normalize_b(nc, st, s_, w_))
    if w + 1 < TW:
        for fc in (0, 1):
            bg_opt.append(lambda fc=fc: _proj_qk_group(nc, st, w + 1, fc))
    if w >= 1:
        for ec in range(CC):
            bg_opt.append(lambda ec=ec: _outproj_group(nc, st, w - 1, ec))

    def bg_slot():
        if bg_must:
            bg_must.pop(0)()
        elif bg_opt:
            bg_opt.pop(0)()

    bg = bg_opt  # leftover drain at window end uses the opt queue

    streams = [
        {"h": 0, "qq": st["QQ"], "kk": st["KK"], "rb": 0,
         "yn_ap": lambda q, n: st["ynA"][0:HD, q:q + n]},
        {"h": 1, "qq": st["QQ"], "kk": st["KK"], "rb": HD,
         "yn_ap": lambda q, n: st["ynA"][HD:128, q:q + n]},
        {"h": 2, "qq": st["QQ2"], "kk": st["KK2"], "rb": 0,
         "yn_ap": lambda q, n: st["ynB"][0:HD, q:q + n]},
    ]
    for s in streams:
        s["y"] = st["psY"].tile([128, 512], F32, tag="y",
                                name=f"y_h{s['h']}_q{w}")

    for bi, kcs in enumerate(batches):
        nb = len(kcs)
        if bi == len(batches) - n_diag_batches:
            while bg_must:
                bg_must.pop(0)()
        # --- scores + exp for all 3 streams; bg slice between h1 and h2 ---
        ebt = {}
        for si, s in enumerate(streams):
            if si == 2:
                bg_slot()
            h = s["h"]
            rb = s["rb"]
            schrau = (SCHRAU and w == TW - 1 and h == 2
                      and all(kc < 4 * w for kc in kcs))
            s_ps = st["psS"].tile([128, EXP_BATCH, 512], F32, tag="s3",
                                  name=f"s_h{h}_q{w}_b{bi}")
            if schrau:
                eti = st["ets"].tile([128, EXP_BATCH, 512], mybir.dt.int16,
                                     tag="et", name=f"e_h{h}_q{w}_b{bi}")
                et = eti.bitcast(mybir.dt.bfloat16)
            else:
                et = st["ets"].tile([128, EXP_BATCH, 512], F16, tag="et",
                                    name=f"e_h{h}_q{w}_b{bi}")
            ebt[h] = et
            js = [max(0, kc - 4 * w) for kc in kcs]
            jw = js if w <= 1 else [min(js)] * nb
            for i in range(nb):
                kc = kcs[i]
                j = jw[i]
                nc.tensor.matmul(
                    s_ps[:, i, 128 * j:512],
                    lhsT=s["kk"][rb:rb + HD, kc * 128:(kc + 1) * 128],
                    rhs=s["qq"][rb:rb + HD, qs + 128 * j:qs + 512],
                    start=True, stop=True,
                )
            if schrau:
                eng = nc.vector if bi % 2 == 0 else nc.gpsimd
                eng.tensor_scalar(
                    out=eti[:, 0:nb, :], in0=s_ps[:, 0:nb, :],
                    scalar1=SCHRAU_A * SCALE, scalar2=SCHRAU_B,
                    op0=mybir.AluOpType.mult, op1=mybir.AluOpType.add)
                continue
            if any(js) and w <= 1:
                # ragged diagonal batch: exp per chunk over exactly the
                # region its score matmul wrote
                for i in range(nb):
                    nc.scalar.activation(
                        out=et[:, i, 128 * js[i]:512],
                        in_=s_ps[:, i, 128 * js[i]:512],
                        func=mybir.ActivationFunctionType.Exp, scale=SCALE,
                    )
            else:
                jm = min(js)
                nc.scalar.activation(
                    out=et[:, 0:nb, 128 * jm:512],
                    in_=s_ps[:, 0:nb, 128 * jm:512],
                    func=mybir.ActivationFunctionType.Exp, scale=SCALE,
                )
            for i in range(nb):
                j = kcs[i] - 4 * w
                if j < 0:
                    continue
                nc.gpsimd.tensor_mul(
                    out=et[:, i, 128 * j:128 * (j + 1)],
                    in0=et[:, i, 128 * j:128 * (j + 1)],
                    in1=st["mask"],
                )
        # --- PV for all 3 streams; bg slice between h1 and h2; on the
        # last round each stream's normalize follows its last PV so the
        # normalize chains overlap the remaining streams' PE work ---
        last_round = bi == len(batches) - 1
        for si, s in enumerate(streams):
            if si == 2:
                bg_slot()
            et = ebt[s["h"]]
            for i in range(nb):
                kc = kcs[i]
                j = max(0, kc - 4 * w)
                idx = bi * EXP_BATCH + i
                nc.tensor.matmul(
                    s["y"][0:HD + 1, 128 * j:512],
                    lhsT=st["vT3"][:, kc, s["h"], 0:HD + 1],
                    rhs=et[:, i, 128 * j:512],
                    start=(idx == 0), stop=(idx == nchunks - 1),
                    skip_group_check=True,
                )
            if last_round:
                _normalize_a(nc, st, s, w)
    if w == TW - 1:
        for s in streams:
            _normalize_b(nc, st, s, w)
    else:
        st["pending_norm"] = [(s, w) for s in streams]

    # leftover background groups
    while bg:
        bg.pop(0)()


def _normalize_a(nc, st, s, w):
    """reciprocal of the sumexp row (column halves on the last window so
    phase B can start earlier; full width otherwise)."""
    h = s["h"]
    y = s["y"]
    sc = st["scr"].tile([128, 512], F32, tag="sc", name=f"sc_h{h}_q{w}")
    s["sc"] = sc
    halves = (0, 256) if w == TW - 1 else (0,)
    wd = 256 if w == TW - 1 else 512
    with nc.allow_low_precision(reason="fp32r == fp32 bits; rounding "
                                "only affects the PE broadcast matmul"):
        for c0 in halves:
            nc.vector.reciprocal(out=sc[0:1, c0:c0 + wd],
                                 in_=y[HD:HD + 1, c0:c0 + wd])


def _normalize_b(nc, st, s, w):
    """broadcast 1/sumexp into the y tile's free partitions 64..127 via a
    K=1 fp32r matmul, then y[0:64] * y[64:128] -> yn (two column halves so
    the output projection can start on the first half early)."""
    qs = w * 512
    h = s["h"]
    y = s["y"]
    sc = s["sc"]
    rb = st["scr"].tile([HD, 512], F32, tag="rbs", name=f"rb_h{h}_q{w}")
    halves = (0, 256) if w == TW - 1 else (0,)
    wd = 256 if w == TW - 1 else 512
    # broadcast 1/sumexp from sc partition 0 to 64 partitions on the GPSIMD
    # engine (SBUF->SBUF; PSUM matmul outputs can't start at partition 64,
    # and the psA bank is contended by background groups)
    for c0 in halves:
        nc.gpsimd.partition_broadcast(rb[:, c0:c0 + wd],
                                      sc[0:1, c0:c0 + wd])
    for c0 in halves:
        nc.vector.tensor_mul(
            out=s["yn_ap"](qs + c0, wd),
            in0=y[0:HD, c0:c0 + wd], in1=rb[:, c0:c0 + wd],
        )


def _prep_core_inputs(c, x, w_qkv, b_qkv, w_out):
    b = c // CPB
    g = c % CPB
    hs = [HPC * g + i for i in range(HPC)]

    qc = [np.arange(h * HD, (h + 1) * HD) for h in hs]
    kc_ = [D + h * HD + np.arange(HD) for h in hs]
    vc = [2 * D + h * HD + np.arange(HD) for h in hs]

    cols = np.concatenate([qc[0], qc[1], qc[2], kc_[2], kc_[0], kc_[1]])
    vcols = np.concatenate(vc)

    xT = np.ascontiguousarray(x[b].T)
    return {
        "xT": np.ascontiguousarray(
            xT.astype(np.float16).reshape(CC, 128, T)),
        "wqk": np.ascontiguousarray(
            w_qkv[:, cols].astype(np.float16).reshape(CC, 128, 3, 128)),
        "bqk": np.ascontiguousarray(
            b_qkv[cols].reshape(3, 128).T.astype(np.float32)),
        "wv": np.ascontiguousarray(
            w_qkv[:, vcols].astype(np.float16).reshape(CC, 128, HPC * HD)),
        "wo01": np.ascontiguousarray(
            w_out[192 * g:192 * g + 128, :].astype(np.float16)),
        "wo2": np.ascontiguousarray(
            w_out[192 * g + 128:192 * g + 192, :].astype(np.float16)),
    }


_NC_CACHE = {}


def get_nc():
    if "nc" not in _NC_CACHE:
        nc = build_bass()
        nc.finalize()
        _NC_CACHE["nc"] = nc
    return _NC_CACHE["nc"]


def kernel(x, w_qkv, b_qkv, w_out, b_out, _run_kwargs=None):
    x = np.asarray(x, dtype=np.float32)
    w_qkv = np.asarray(w_qkv, dtype=np.float32)
    b_qkv = np.asarray(b_qkv, dtype=np.float32)
    w_out = np.asarray(w_out, dtype=np.float32)
    b_out = np.asarray(b_out, dtype=np.float32)

    nc = get_nc()
    in_maps = [_prep_core_inputs(c, x, w_qkv, b_qkv, w_out)
               for c in range(NCORES)]
    kwargs = dict(_run_kwargs or {})
    res = run_bass_kernel_spmd(nc, in_maps, core_ids=list(range(NCORES)),
                               **kwargs)
    if kwargs:
        _NC_CACHE["last_results"] = res

    bv_corr = b_qkv[2 * D:3 * D] @ w_out  # [D]
    out = np.zeros((B, T, D), dtype=np.float32)
    for b in range(B):
        acc = np.zeros((T, D), dtype=np.float32)
        for g in range(CPB):
            acc += np.asarray(res.results[b * CPB + g]["outT"]
                              ).astype(np.float32).T
        out[b] = acc + (b_out + bv_corr)[None, :]
    return out


if __name__ == "__main__":
    nc = build_bass()
    print("built OK")


# revision 7
# speedup vs baseline: 1.1080x; 1.0358x over previous
"""Causal self-attention (B=2, T=2048, D=768, H=12) on 8 TRN2 NeuronCores.

Sharding: tensor-parallel over (batch, head) pairs; 3 heads per core, one
batch per 4-core group. All on-device tensors are fp16 (same PE/DVE cost as
bf16 under the cost model, 8x less rounding noise). Per 512-token window:
q/k/v projection, then causal attention with the scores->exp->PV chain
software-pipelined in "rounds" of 2-k-chunk batches across the 3 head
streams, with next-window projection and previous-window output-projection
matmul groups interleaved between rounds to keep the PE busy while ScalarE
exp latency drains. QKV bias is folded into the PSUM->SBUF evacuation
(DVE tensor_scalar). Host sums the 4 partial outputs per batch and adds
b_out (+ the v-bias correction through w_out).
"""

import numpy as np

import concourse.bass as bass
import concourse.bacc as bacc
import concourse.mybir as mybir
import concourse.tile as tile
from concourse import library_config
from concourse.masks import make_upper_triangular
from concourse.bass_utils import run_bass_kernel_spmd

B, T, D, H, HD = 2, 2048, 768, 12, 64
NCORES = 8
HPC = 3            # heads per core
CPB = NCORES // B  # cores per batch = 4
CC = D // 128      # d_model chunks of 128 = 6
TW = T // 512      # token windows of 512 = 4
KC = T // 128      # k chunks of 128 = 16
SCALE = 1.0 / float(np.sqrt(HD))

F16 = mybir.dt.float16
F32 = mybir.dt.float32
F32R = mybir.dt.float32r

EXP_BATCH = 2  # k-chunks per exp call / per s-tile (PSUM tile = 2 banks)

# Schraudolph fast-exp on DVE/Pool for stream h2's below-diagonal batches in
# late windows (relieves the ScalarE bottleneck there). exp(x) ~ bf16 bitcast
# of int16(A*x + B); ~2.4% RMS approximation error on ~11% of the attention
# weights => ~8e-3 end-to-end rel err (budget 2e-2).
SCHRAU = False
SCHRAU_A = 128.0 / np.log(2.0)
SCHRAU_B = float(127 << 7) - 7.5


def build_bass():
    nc = bacc.Bacc(None, target_bir_lowering=False)

    xT = nc.dram_tensor("xT", [CC, 128, T], F16, kind="ExternalInput")
    wqk = nc.dram_tensor("wqk", [CC, 128, 3, 128], F16, kind="ExternalInput")
    bqk = nc.dram_tensor("bqk", [128, 3], F32, kind="ExternalInput")
    wv = nc.dram_tensor("wv", [CC, 128, HPC * HD], F16, kind="ExternalInput")
    wo01d = nc.dram_tensor("wo01", [128, D], F16, kind="ExternalInput")
    wo2d = nc.dram_tensor("wo2", [HD, D], F16, kind="ExternalInput")
    outT = nc.dram_tensor("outT", [D, T], F16, kind="ExternalOutput")

    with tile.TileContext(nc) as tc:
        with (
            tc.tile_pool(name="big", bufs=1) as big,
            tc.tile_pool(name="ets", bufs=6) as ets,
            tc.tile_pool(name="scr", bufs=3) as scr,
            tc.tile_pool(name="outs", bufs=4) as outs,
            tc.tile_pool(name="psS", bufs=2, space="PSUM") as psS,
            tc.tile_pool(name="psY", bufs=3, space="PSUM") as psY,
            tc.tile_pool(name="psA", bufs=1, space="PSUM") as psA,
        ):
            # ---- SBUF persistent tiles ----
            wqks = big.tile([128, CC, 3, 128], F16, tag="wqk")
            wvs = big.tile([128, CC, HPC * HD], F16, tag="wv")
            xTs = big.tile([128, CC, T], F16, tag="xT")
            bqks = big.tile([128, 3], F32, tag="bqk")
            wos01 = big.tile([128, D], F16, tag="wo01")
            wos2 = big.tile([HD, D], F16, tag="wo2")
            QQ = big.tile([128, T], F16, tag="QQ")
            KK = big.tile([128, T], F16, tag="KK")
            QQ2 = big.tile([HD, T], F16, tag="QQ2")
            KK2 = big.tile([HD, T], F16, tag="KK2")
            # token-major V (+ ones column at 64): [128, kc, h, 66]
            vT3 = big.tile([128, KC, HPC, 66], F16, tag="vT3")
            ynA = big.tile([128, T], F16, tag="ynA")
            ynB = big.tile([HD, T], F16, tag="ynB")
            mask_tri = big.tile([128, 128], F16, tag="mask")
            ones_t = big.tile([128, HD], F32R, tag="ones")

            # PE p-state warm-up: a dense run of ~free N=1 matmuls starts
            # the tensor engine's ramp clock during the input-DMA wait so
            # the real matmuls reach full clock ~2us earlier; the dummy Exp
            # pulls the activation-table load (1.3us) off the first real
            # exp's critical path.
            wtiny = big.tile([1, 8], F16, tag="wtiny")
            nc.vector.memset(wtiny, 0.5)
            nc.scalar.activation(out=wtiny[0:1, 4:8], in_=wtiny[0:1, 0:4],
                                 func=mybir.ActivationFunctionType.Exp)
            s_warm = psS.tile([128, EXP_BATCH, 512], F32, tag="s3",
                              name="s_warm")
            warm_cols = 1024
            for i in range(550):
                j = i % warm_cols
                nc.tensor.matmul(
                    s_warm[0:1, j // 512, j % 512:j % 512 + 1],
                    lhsT=wtiny[0:1, 0:1], rhs=wtiny[0:1, 0:1],
                    start=True, stop=True, skip_group_check=True)

            # ---- input DMAs: window-0 criticals first. wqk goes through
            # the SP HWDGE queue while xT window-0 chunks go through the
            # Pool SWDGE queue -- two parallel descriptor-generation paths.
            nc.gpsimd.dma_start(out=xTs[:, 0, 0:512], in_=xT[0, :, 0:512])
            nc.gpsimd.dma_start(out=xTs[:, 3, 0:512], in_=xT[3, :, 0:512])
            nc.gpsimd.dma_start(out=xTs[:, 4, 0:512], in_=xT[4, :, 0:512])
            nc.gpsimd.dma_start(out=xTs[:, 5, 0:512], in_=xT[5, :, 0:512])
            nc.gpsimd.dma_start(out=wqks[:, 5], in_=wqk[5])
            # the GPSIMD ucode library carrying partition_broadcast loads
            # after the startup DMAs so it doesn't gate them
            nc.gpsimd.load_library(library_config.proxy)
            nc.sync.dma_start(out=wqks[:, 0, 0:1], in_=wqk[0, :, 0:1])
            nc.sync.dma_start(out=xTs[:, 1, 0:512], in_=xT[1, :, 0:512])
            nc.sync.dma_start(out=wqks[:, 0, 1:3], in_=wqk[0, :, 1:3])
            nc.sync.dma_start(out=wqks[:, 1], in_=wqk[1])
            nc.sync.dma_start(out=xTs[:, 2, 0:512], in_=xT[2, :, 0:512])
            nc.sync.dma_start(out=wqks[:, 2], in_=wqk[2])
            nc.sync.dma_start(out=wvs, in_=wv.rearrange("c p f -> p c f"))
            nc.sync.dma_start(out=wqks[:, 3], in_=wqk[3])
            nc.sync.dma_start(out=wqks[:, 4], in_=wqk[4])
            nc.sync.dma_start(out=bqks, in_=bqk[:, :])
            nc.sync.dma_start(out=wos01, in_=wo01d[:, :])
            nc.sync.dma_start(out=wos2, in_=wo2d[:, :])
            for cc in range(CC):
                nc.sync.dma_start(out=xTs[:, cc, 512:T],
                                  in_=xT[cc, :, 512:T])

            # ---- constants ----
            make_upper_triangular(nc, mask_tri, val=1.0, diag=True)
            ones_stage = big.tile([128, HD], F32, tag="ones_stage")
            nc.vector.memset(ones_stage, 1.0)
            with nc.allow_low_precision(reason="fp32r ones for normalizer "
                                        "broadcast matmul"):
                nc.vector.tensor_copy(out=ones_t, in_=ones_stage)
            for h in range(HPC):
                nc.gpsimd.memset(vT3[:, :, h, HD:HD + 1], 1.0)

            st = {
                "pending_norm": [],
                "wqks": wqks, "wvs": wvs, "xTs": xTs, "bqks": bqks,
                "wos01": wos01, "wos2": wos2, "QQ": QQ, "KK": KK,
                "QQ2": QQ2, "KK2": KK2, "vT3": vT3, "ynA": ynA, "ynB": ynB,
                "mask": mask_tri, "ones": ones_t,
                "psS": psS, "psY": psY, "psA": psA,
                "ets": ets, "scr": scr, "outs": outs, "outT": outT,
            }

            # prologue: window-0 q/k projection, cc-major across three
            # psY accumulators so each arriving x chunk is consumed
            # immediately; V chunks 0/1 here, 2/3 ride window 0's bg queue.
            paccs = [psY.tile([128, 512], F32, tag="y", name=f"pacc_f{fc}")
                     for fc in range(3)]
            for cc in range(CC):
                for fc in range(3):
                    nc.tensor.matmul(
                        paccs[fc],
                        lhsT=wqks[:, cc, fc, :],
                        rhs=xTs[:, cc, 0:512],
                        start=(cc == 0), stop=(cc == CC - 1),
                    )
            for fc in range(3):
                _evac_qk(nc, st, 0, fc, paccs[fc])
            _proj_v_chunk(nc, st, 0)
            _proj_v_chunk(nc, st, 1, acc="y")

            for w in range(TW):
                _attn_window(nc, st, w)

            # epilogue: output projection for the last window, split-phase:
            # the h0/h1 contraction runs over 6 parallel accumulators
            # (borrowing the now-idle psS/psY banks) while the h2 stream's
            # normalize drains, then the h2 matmuls close each group.
            qs = (TW - 1) * 512
            opst = []
            for ec in range(CC):
                pool, tag = [(psY, "y"), (psY, "y"), (psY, "y"),
                             (psS, "s3"), (psS, "s3"), (psA, "acc")][ec]
                opst.append(pool.tile([128, 512], F32, tag=tag,
                                      name=f"opse_{ec}"))
            osb6e = outs.tile([128, CC, 512], F16, tag="osb",
                              name="osb_epi")
            for ec in range(CC):
                for c0 in (0, 256):
                    nc.tensor.matmul(
                        opst[ec][:, c0:c0 + 256],
                        lhsT=wos01[:, ec * 128:(ec + 1) * 128],
                        rhs=ynA[:, qs + c0:qs + c0 + 256],
                        start=(c0 == 0), stop=False, skip_group_check=True,
                    )
                for c0 in (0, 256):
                    nc.tensor.matmul(
                        opst[ec][:, c0:c0 + 256],
                        lhsT=wos2[:, ec * 128:(ec + 1) * 128],
                        rhs=ynB[:, qs + c0:qs + c0 + 256],
                        start=False, stop=True, skip_group_check=True,
                    )
                if ec % 2 == 0:
                    nc.vector.tensor_copy(out=osb6e[:, ec, :], in_=opst[ec])
                else:
                    nc.scalar.copy(out=osb6e[:, ec, :], in_=opst[ec])
                if ec % 2 == 1:
                    nc.sync.dma_start(
                        out=outT[(ec - 1) * 128:(ec + 1) * 128,
                                 qs:qs + 512].rearrange(
                            "(e p) c -> p e c", e=2),
                        in_=osb6e[:, ec - 1:ec + 1, :],
                    )
    return nc


def _proj_qk_window(nc, st, w):
    """q/k projection for token window w: 3 fc groups of 6 matmuls each,
    bias folded into the DVE evacuation."""
    ts_ = w * 512
    for fc in range(3):
        _proj_qk_group(nc, st, w, fc)
    del ts_


def _proj_qk_group(nc, st, w, fc, acc="acc"):
    ts_ = w * 512
    pool = st["psY"] if acc == "y" else st["psA"]
    ps = pool.tile([128, 512], F32, tag=acc, name=f"ps_f{fc}_t{w}")
    for cc in range(CC):
        nc.tensor.matmul(
            ps,
            lhsT=st["wqks"][:, cc, fc, :],
            rhs=st["xTs"][:, cc, ts_:ts_ + 512],
            start=(cc == 0), stop=(cc == CC - 1),
        )
    _evac_qk(nc, st, w, fc, ps)


def _evac_qk(nc, st, w, fc, ps):
    ts_ = w * 512
    # evacuate with bias add (per-partition scalar).
    # fc0 = [h0q|h1q] -> QQ; fc1 = [h2q|h2k] -> QQ2/KK2; fc2 = [h0k|h1k] -> KK
    if fc == 0:
        nc.vector.tensor_scalar(
            out=st["QQ"][:, ts_:ts_ + 512], in0=ps,
            scalar1=st["bqks"][:, fc:fc + 1], scalar2=None,
            op0=mybir.AluOpType.add)
    elif fc == 2:
        nc.vector.tensor_scalar(
            out=st["KK"][:, ts_:ts_ + 512], in0=ps,
            scalar1=st["bqks"][:, fc:fc + 1], scalar2=None,
            op0=mybir.AluOpType.add)
    else:
        nc.vector.tensor_scalar(
            out=st["QQ2"][:, ts_:ts_ + 512], in0=ps[0:HD, :],
            scalar1=st["bqks"][0:HD, fc:fc + 1], scalar2=None,
            op0=mybir.AluOpType.add)
        nc.vector.tensor_scalar(
            out=st["KK2"][:, ts_:ts_ + 512], in0=ps[HD:128, :],
            scalar1=st["bqks"][HD:128, fc:fc + 1], scalar2=None,
            op0=mybir.AluOpType.add)


def _proj_v_chunk(nc, st, tc_i, acc="acc"):
    """token-major V projection for one 128-token chunk."""
    pool = st["psY"] if acc == "y" else st["psA"]
    psv = pool.tile([128, 512], F32, tag=acc, name=f"psv_{tc_i}")
    for cc in range(CC):
        nc.tensor.matmul(
            psv[:, 0:HPC * HD],
            lhsT=st["xTs"][:, cc, tc_i * 128:(tc_i + 1) * 128],
            rhs=st["wvs"][:, cc, :],
            start=(cc == 0), stop=(cc == CC - 1),
        )
    nc.vector.tensor_copy(
        out=st["vT3"][:, tc_i, :, 0:HD],
        in_=psv[:, 0:HPC * HD].rearrange("p (h d) -> p h d", h=HPC),
    )


def _outproj_group(nc, st, w, ec):
    qs = w * 512
    act_ok = w < TW - 2  # evacs run in window w+1; Act has slack if w+1<=2
    if ec == 0:
        st["osb6"] = st["outs"].tile([128, CC, 512], F16, tag="osb",
                                     name=f"osb_q{w}")
    ops = st["psA"].tile([128, 512], F32, tag="acc", name=f"ops_e{ec}_q{w}")
    nc.tensor.matmul(
        ops,
        lhsT=st["wos01"][:, ec * 128:(ec + 1) * 128],
        rhs=st["ynA"][:, qs:qs + 512],
        start=True, stop=False,
    )
    nc.tensor.matmul(
        ops,
        lhsT=st["wos2"][:, ec * 128:(ec + 1) * 128],
        rhs=st["ynB"][:, qs:qs + 512],
        start=False, stop=True,
    )
    nc.vector.tensor_copy(out=st["osb6"][:, ec, :], in_=ops)
    if ec == CC - 1:
        nc.sync.dma_start(
            out=st["outT"][:, qs:qs + 512].rearrange(
                "(e p) c -> p e c", e=CC),
            in_=st["osb6"],
        )


def _attn_window(nc, st, w):
    """Attention for q-window w across the 3 head streams, with background
    PE work (next-window projection, previous-window outproj) interleaved
    between score/PV rounds."""
    qs = w * 512
    nchunks = 4 * (w + 1)
    # below-diagonal chunks first (descending, so the first PV write is the
    # full column range), diagonal chunks last: the window's own K-side
    # projection (fc2) and V chunks then slide into this window's early
    # rounds instead of crowding the previous one.
    kc_order = list(range(4 * w))[::-1] + list(range(4 * w, nchunks))
    batches = [kc_order[i:i + EXP_BATCH]
               for i in range(0, nchunks, EXP_BATCH)]
    n_diag_batches = 2

    # bg_must: work that must land before this window's diagonal rounds.
    # bg_opt: deferrable work (previous window's normalize phase B and
    # outproj, next window's Q-side projection).
    bg_must = []
    if w == 0:
        for j in (2, 3):
            bg_must.append(lambda j=j: _proj_v_chunk(nc, st, j))
    else:
        bg_must.append(lambda: _proj_qk_group(nc, st, w, 2))
        for j in range(4):
            bg_must.append(lambda j=j: _proj_v_chunk(nc, st, 4 * w + j))
    bg_opt = []
    for s_, w_ in st.pop("pending_norm", []):
        bg_opt.append(lambda s_=s_, w_=w_: _normalize_b(nc, st, s_, w_))
    if w + 1 < TW:
        for fc in (0, 1):
            bg_opt.append(lambda fc=fc: _proj_qk_group(nc, st, w + 1, fc))
    if w >= 1:
        for ec in range(CC):
            bg_opt.append(lambda ec=ec: _outproj_group(nc, st, w - 1, ec))

    def bg_slot():
        if bg_must:
            bg_must.pop(0)()
        elif bg_opt:
            bg_opt.pop(0)()

    bg = bg_opt  # leftover drain at window end uses the opt queue

    streams = [
        {"h": 0, "qq": st["QQ"], "kk": st["KK"], "rb": 0,
         "yn_ap": lambda q, n: st["ynA"][0:HD, q:q + n]},
        {"h": 1, "qq": st["QQ"], "kk": st["KK"], "rb": HD,
         "yn_ap": lambda q, n: st["ynA"][HD:128, q:q + n]},
        {"h": 2, "qq": st["QQ2"], "kk": st["KK2"], "rb": 0,
         "yn_ap": lambda q, n: st["ynB"][0:HD, q:q + n]},
    ]
    for s in streams:
        s["y"] = st["psY"].tile([128, 512], F32, tag="y",
                                name=f"y_h{s['h']}_q{w}")

    for bi, kcs in enumerate(batches):
        nb = len(kcs)
        if bi == len(batches) - n_diag_batches:
            while bg_must:
                bg_must.pop(0)()
        # --- scores + exp for all 3 streams; bg slice between h1 and h2 ---
        ebt = {}
        for si, s in enumerate(streams):
            if si == 2:
                bg_slot()
            h = s["h"]
            rb = s["rb"]
            schrau = (SCHRAU and w == TW - 1 and h == 2
                      and all(kc < 4 * w for kc in kcs))
            s_ps = st["psS"].tile([128, EXP_BATCH, 512], F32, tag="s3",
                                  name=f"s_h{h}_q{w}_b{bi}")
            if schrau:
                eti = st["ets"].tile([128, EXP_BATCH, 512], mybir.dt.int16,
                                     tag="et", name=f"e_h{h}_q{w}_b{bi}")
                et = eti.bitcast(mybir.dt.bfloat16)
            else:
                et = st["ets"].tile([128, EXP_BATCH, 512], F16, tag="et",
                                    name=f"e_h{h}_q{w}_b{bi}")
            ebt[h] = et
            js = [max(0, kc - 4 * w) for kc in kcs]
            jw = js if w <= 1 else [min(js)] * nb
            for i in range(nb):
                kc = kcs[i]
                j = jw[i]
                nc.tensor.matmul(
                    s_ps[:, i, 128 * j:512],
                    lhsT=s["kk"][rb:rb + HD, kc * 128:(kc + 1) * 128],
                    rhs=s["qq"][rb:rb + HD, qs + 128 * j:qs + 512],
                    start=True, stop=True,
                )
            if schrau:
                nc.vector.tensor_scalar(
                    out=eti[:, 0:nb, :], in0=s_ps[:, 0:nb, :],
                    scalar1=SCHRAU_A * SCALE, scalar2=SCHRAU_B,
                    op0=mybir.AluOpType.mult, op1=mybir.AluOpType.add)
                continue
            if any(js) and w <= 1:
                # ragged diagonal batch: exp per chunk over exactly the
                # region its score matmul wrote
                for i in range(nb):
                    nc.scalar.activation(
                        out=et[:, i, 128 * js[i]:512],
                        in_=s_ps[:, i, 128 * js[i]:512],
                        func=mybir.ActivationFunctionType.Exp, scale=SCALE,
                    )
            else:
                jm = min(js)
                nc.scalar.activation(
                    out=et[:, 0:nb, 128 * jm:512],
                    in_=s_ps[:, 0:nb, 128 * jm:512],
                    func=mybir.ActivationFunctionType.Exp, scale=SCALE,
                )
            for i in range(nb):
                j = kcs[i] - 4 * w
                if j < 0:
                    continue
                nc.gpsimd.tensor_mul(
                    out=et[:, i, 128 * j:128 * (j + 1)],
                    in0=et[:, i, 128 * j:128 * (j + 1)],
                    in1=st["mask"],
                )
        # --- PV for all 3 streams; bg slice between h1 and h2; on the
        # last round each stream's normalize follows its last PV so the
        # normalize chains overlap the remaining streams' PE work ---
        last_round = bi == len(batches) - 1
        next_last = bi == len(batches) - 2
        for si, s in enumerate(streams):
            if si == 2:
                bg_slot()
            et = ebt[s["h"]]
            for i in range(nb):
                kc = kcs[i]
                j = max(0, kc - 4 * w)
                idx = bi * EXP_BATCH + i
                nc.tensor.matmul(
                    s["y"][0:HD + 1, 128 * j:512],
                    lhsT=st["vT3"][:, kc, s["h"], 0:HD + 1],
                    rhs=et[:, i, 128 * j:512],
                    start=(idx == 0), stop=(idx == nchunks - 1),
                    skip_group_check=True,
                )
            if w == TW - 1:
                # the diagonal chunks only touch ascending column ranges, so
                # sumexp[0:256] is final one batch early: run the normalize
                # chain per column half as it becomes final, overlapping the
                # last batch's score/exp/PV work
                if next_last:
                    _normalize_half(nc, st, s, w, 0)
                elif last_round:
                    _normalize_half(nc, st, s, w, 256)
            elif last_round:
                _normalize_a(nc, st, s, w)
    if w != TW - 1:
        st["pending_norm"] = [(s, w) for s in streams]

    # leftover background groups
    while bg:
        bg.pop(0)()


def _normalize_half(nc, st, s, w, c0):
    """full normalize chain for one 256-wide column half (last window)."""
    qs = w * 512
    h = s["h"]
    y = s["y"]
    if c0 == 0:
        s["sc"] = st["scr"].tile([128, 512], F32, tag="sc",
                                 name=f"sc_h{h}_q{w}")
        s["rbt"] = st["scr"].tile([HD, 512], F32, tag="rbs",
                                  name=f"rb_h{h}_q{w}")
    sc, rb = s["sc"], s["rbt"]
    nc.vector.reciprocal(out=sc[0:1, c0:c0 + 256],
                         in_=y[HD:HD + 1, c0:c0 + 256])
    nc.gpsimd.partition_broadcast(rb[:, c0:c0 + 256], sc[0:1, c0:c0 + 256])
    nc.vector.tensor_mul(
        out=s["yn_ap"](qs + c0, 256),
        in0=y[0:HD, c0:c0 + 256], in1=rb[:, c0:c0 + 256],
    )


def _normalize_a(nc, st, s, w):
    """reciprocal of the sumexp row (column halves on the last window so
    phase B can start earlier; full width otherwise)."""
    h = s["h"]
    y = s["y"]
    sc = st["scr"].tile([128, 512], F32, tag="sc", name=f"sc_h{h}_q{w}")
    s["sc"] = sc
    halves = (0, 256) if w == TW - 1 else (0,)
    wd = 256 if w == TW - 1 else 512
    with nc.allow_low_precision(reason="fp32r == fp32 bits; rounding "
                                "only affects the PE broadcast matmul"):
        for c0 in halves:
            nc.vector.reciprocal(out=sc[0:1, c0:c0 + wd],
                                 in_=y[HD:HD + 1, c0:c0 + wd])


def _normalize_b(nc, st, s, w):
    """broadcast 1/sumexp into the y tile's free partitions 64..127 via a
    K=1 fp32r matmul, then y[0:64] * y[64:128] -> yn (two column halves so
    the output projection can start on the first half early)."""
    qs = w * 512
    h = s["h"]
    y = s["y"]
    sc = s["sc"]
    rb = st["scr"].tile([HD, 512], F32, tag="rbs", name=f"rb_h{h}_q{w}")
    halves = (0, 256) if w == TW - 1 else (0,)
    wd = 256 if w == TW - 1 else 512
    # broadcast 1/sumexp from sc partition 0 to 64 partitions on the GPSIMD
    # engine (SBUF->SBUF; PSUM matmul outputs can't start at partition 64,
    # and the psA bank is contended by background groups)
    for c0 in halves:
        nc.gpsimd.partition_broadcast(rb[:, c0:c0 + wd],
                                      sc[0:1, c0:c0 + wd])
    for c0 in halves:
        nc.vector.tensor_mul(
            out=s["yn_ap"](qs + c0, wd),
            in0=y[0:HD, c0:c0 + wd], in1=rb[:, c0:c0 + wd],
        )


def _prep_core_inputs(c, x, w_qkv, b_qkv, w_out):
    b = c // CPB
    g = c % CPB
    hs = [HPC * g + i for i in range(HPC)]

    qc = [np.arange(h * HD, (h + 1) * HD) for h in hs]
    kc_ = [D + h * HD + np.arange(HD) for h in hs]
    vc = [2 * D + h * HD + np.arange(HD) for h in hs]

    cols = np.concatenate([qc[0], qc[1], qc[2], kc_[2], kc_[0], kc_[1]])
    vcols = np.concatenate(vc)

    xT = np.ascontiguousarray(x[b].T)
    return {
        "xT": np.ascontiguousarray(
            xT.astype(np.float16).reshape(CC, 128, T)),
        "wqk": np.ascontiguousarray(
            w_qkv[:, cols].astype(np.float16).reshape(CC, 128, 3, 128)),
        "bqk": np.ascontiguousarray(
            b_qkv[cols].reshape(3, 128).T.astype(np.float32)),
        "wv": np.ascontiguousarray(
            w_qkv[:, vcols].astype(np.float16).reshape(CC, 128, HPC * HD)),
        "wo01": np.ascontiguousarray(
            w_out[192 * g:192 * g + 128, :].astype(np.float16)),
        "wo2": np.ascontiguousarray(
            w_out[192 * g + 128:192 * g + 192, :].astype(np.float16)),
    }


_NC_CACHE = {}


def get_nc():
    if "nc" not in _NC_CACHE:
        nc = build_bass()
        nc.finalize()
        _NC_CACHE["nc"] = nc
    return _NC_CACHE["nc"]


def kernel(x, w_qkv, b_qkv, w_out, b_out, _run_kwargs=None):
    x = np.asarray(x, dtype=np.float32)
    w_qkv = np.asarray(w_qkv, dtype=np.float32)
    b_qkv = np.asarray(b_qkv, dtype=np.float32)
    w_out = np.asarray(w_out, dtype=np.float32)
    b_out = np.asarray(b_out, dtype=np.float32)

    nc = get_nc()
    in_maps = [_prep_core_inputs(c, x, w_qkv, b_qkv, w_out)
               for c in range(NCORES)]
    kwargs = dict(_run_kwargs or {})
    res = run_bass_kernel_spmd(nc, in_maps, core_ids=list(range(NCORES)),
                               **kwargs)
    if kwargs:
        _NC_CACHE["last_results"] = res

    bv_corr = b_qkv[2 * D:3 * D] @ w_out  # [D]
    out = np.zeros((B, T, D), dtype=np.float32)
    for b in range(B):
        acc = np.zeros((T, D), dtype=np.float32)
        for g in range(CPB):
            acc += np.asarray(res.results[b * CPB + g]["outT"]
                              ).astype(np.float32).T
        out[b] = acc + (b_out + bv_corr)[None, :]
    return out


if __name__ == "__main__":
    nc = build_bass()
    print("built OK")


# revision 8
# speedup vs baseline: 1.1085x; 1.0004x over previous
"""Causal self-attention (B=2, T=2048, D=768, H=12) on 8 TRN2 NeuronCores.

Sharding: tensor-parallel over (batch, head) pairs; 3 heads per core, one
batch per 4-core group. All on-device tensors are fp16 (same PE/DVE cost as
bf16 under the cost model, 8x less rounding noise). Per 512-token window:
q/k/v projection, then causal attention with the scores->exp->PV chain
software-pipelined in "rounds" of 2-k-chunk batches across the 3 head
streams, with next-window projection and previous-window output-projection
matmul groups interleaved between rounds to keep the PE busy while ScalarE
exp latency drains. QKV bias is folded into the PSUM->SBUF evacuation
(DVE tensor_scalar). Host sums the 4 partial outputs per batch and adds
b_out (+ the v-bias correction through w_out).
"""

import numpy as np

import concourse.bass as bass
import concourse.bacc as bacc
import concourse.mybir as mybir
import concourse.tile as tile
from concourse import library_config
from concourse.masks import make_upper_triangular
from concourse.bass_utils import run_bass_kernel_spmd

B, T, D, H, HD = 2, 2048, 768, 12, 64
NCORES = 8
HPC = 3            # heads per core
CPB = NCORES // B  # cores per batch = 4
CC = D // 128      # d_model chunks of 128 = 6
TW = T // 512      # token windows of 512 = 4
KC = T // 128      # k chunks of 128 = 16
SCALE = 1.0 / float(np.sqrt(HD))

F16 = mybir.dt.float16
F32 = mybir.dt.float32
F32R = mybir.dt.float32r

EXP_BATCH = 2  # k-chunks per exp call / per s-tile (PSUM tile = 2 banks)

# Schraudolph fast-exp on DVE/Pool for stream h2's below-diagonal batches in
# late windows (relieves the ScalarE bottleneck there). exp(x) ~ bf16 bitcast
# of int16(A*x + B); ~2.4% RMS approximation error on ~11% of the attention
# weights => ~8e-3 end-to-end rel err (budget 2e-2).
SCHRAU = False
SCHRAU_A = 128.0 / np.log(2.0)
SCHRAU_B = float(127 << 7) - 7.5


def build_bass():
    nc = bacc.Bacc(None, target_bir_lowering=False)

    xT = nc.dram_tensor("xT", [CC, 128, T], F16, kind="ExternalInput")
    wqk = nc.dram_tensor("wqk", [CC, 128, 3, 128], F16, kind="ExternalInput")
    bqk = nc.dram_tensor("bqk", [128, 3], F32, kind="ExternalInput")
    wv = nc.dram_tensor("wv", [CC, 128, HPC * HD], F16, kind="ExternalInput")
    wo01d = nc.dram_tensor("wo01", [128, D], F16, kind="ExternalInput")
    wo2d = nc.dram_tensor("wo2", [HD, D], F16, kind="ExternalInput")
    outT = nc.dram_tensor("outT", [D, T], F16, kind="ExternalOutput")

    with tile.TileContext(nc) as tc:
        with (
            tc.tile_pool(name="big", bufs=1) as big,
            tc.tile_pool(name="ets", bufs=6) as ets,
            tc.tile_pool(name="scr", bufs=3) as scr,
            tc.tile_pool(name="outs", bufs=4) as outs,
            tc.tile_pool(name="psS", bufs=2, space="PSUM") as psS,
            tc.tile_pool(name="psY", bufs=3, space="PSUM") as psY,
            tc.tile_pool(name="psA", bufs=1, space="PSUM") as psA,
        ):
            # ---- SBUF persistent tiles ----
            wqks = big.tile([128, CC, 3, 128], F16, tag="wqk")
            wvs = big.tile([128, CC, HPC * HD], F16, tag="wv")
            xTs = big.tile([128, CC, T], F16, tag="xT")
            bqks = big.tile([128, 3], F32, tag="bqk")
            wos01 = big.tile([128, D], F16, tag="wo01")
            wos2 = big.tile([HD, D], F16, tag="wo2")
            QQ = big.tile([128, T], F16, tag="QQ")
            KK = big.tile([128, T], F16, tag="KK")
            QQ2 = big.tile([HD, T], F16, tag="QQ2")
            KK2 = big.tile([HD, T], F16, tag="KK2")
            # token-major V (+ ones column at 64): [128, kc, h, 66]
            vT3 = big.tile([128, KC, HPC, 66], F16, tag="vT3")
            ynA = big.tile([128, T], F16, tag="ynA")
            ynB = big.tile([HD, T], F16, tag="ynB")
            mask_tri = big.tile([128, 128], F16, tag="mask")
            ones_t = big.tile([128, HD], F32R, tag="ones")

            # PE p-state warm-up: a dense run of ~free N=1 matmuls starts
            # the tensor engine's ramp clock during the input-DMA wait so
            # the real matmuls reach full clock ~2us earlier; the dummy Exp
            # pulls the activation-table load (1.3us) off the first real
            # exp's critical path.
            wtiny = big.tile([1, 8], F16, tag="wtiny")
            nc.vector.memset(wtiny, 0.5)
            nc.scalar.activation(out=wtiny[0:1, 4:8], in_=wtiny[0:1, 0:4],
                                 func=mybir.ActivationFunctionType.Exp)
            s_warm = psS.tile([128, EXP_BATCH, 512], F32, tag="s3",
                              name="s_warm")
            warm_cols = 1024
            for i in range(550):
                j = i % warm_cols
                nc.tensor.matmul(
                    s_warm[0:1, j // 512, j % 512:j % 512 + 1],
                    lhsT=wtiny[0:1, 0:1], rhs=wtiny[0:1, 0:1],
                    start=True, stop=True, skip_group_check=True)

            # ---- input DMAs: window-0 criticals first. wqk goes through
            # the SP HWDGE queue while xT window-0 chunks go through the
            # Pool SWDGE queue -- two parallel descriptor-generation paths.
            nc.gpsimd.dma_start(out=wqks[:, 0, 0:1], in_=wqk[0, :, 0:1])
            nc.gpsimd.dma_start(out=xTs[:, 3, 0:512], in_=xT[3, :, 0:512])
            nc.gpsimd.dma_start(out=xTs[:, 4, 0:512], in_=xT[4, :, 0:512])
            nc.gpsimd.dma_start(out=xTs[:, 5, 0:512], in_=xT[5, :, 0:512])
            nc.gpsimd.dma_start(out=wqks[:, 5], in_=wqk[5])
            # the GPSIMD ucode library carrying partition_broadcast loads
            # after the startup DMAs so it doesn't gate them
            nc.gpsimd.load_library(library_config.proxy)
            nc.sync.dma_start(out=xTs[:, 0, 0:512], in_=xT[0, :, 0:512])
            nc.sync.dma_start(out=xTs[:, 1, 0:512], in_=xT[1, :, 0:512])
            nc.sync.dma_start(out=wqks[:, 0, 1:3], in_=wqk[0, :, 1:3])
            nc.sync.dma_start(out=wqks[:, 1], in_=wqk[1])
            nc.sync.dma_start(out=xTs[:, 2, 0:512], in_=xT[2, :, 0:512])
            nc.sync.dma_start(out=wqks[:, 2], in_=wqk[2])
            nc.sync.dma_start(out=wvs, in_=wv.rearrange("c p f -> p c f"))
            nc.sync.dma_start(out=wqks[:, 3], in_=wqk[3])
            nc.sync.dma_start(out=wqks[:, 4], in_=wqk[4])
            nc.sync.dma_start(out=bqks, in_=bqk[:, :])
            nc.sync.dma_start(out=wos01, in_=wo01d[:, :])
            nc.sync.dma_start(out=wos2, in_=wo2d[:, :])
            for cc in range(CC):
                nc.sync.dma_start(out=xTs[:, cc, 512:T],
                                  in_=xT[cc, :, 512:T])

            # ---- constants ----
            make_upper_triangular(nc, mask_tri, val=1.0, diag=True)
            ones_stage = big.tile([128, HD], F32, tag="ones_stage")
            nc.vector.memset(ones_stage, 1.0)
            with nc.allow_low_precision(reason="fp32r ones for normalizer "
                                        "broadcast matmul"):
                nc.vector.tensor_copy(out=ones_t, in_=ones_stage)
            for h in range(HPC):
                nc.gpsimd.memset(vT3[:, :, h, HD:HD + 1], 1.0)

            st = {
                "pending_norm": [],
                "wqks": wqks, "wvs": wvs, "xTs": xTs, "bqks": bqks,
                "wos01": wos01, "wos2": wos2, "QQ": QQ, "KK": KK,
                "QQ2": QQ2, "KK2": KK2, "vT3": vT3, "ynA": ynA, "ynB": ynB,
                "mask": mask_tri, "ones": ones_t,
                "psS": psS, "psY": psY, "psA": psA,
                "ets": ets, "scr": scr, "outs": outs, "outT": outT,
            }

            # prologue: window-0 q/k projection, cc-major across three
            # psY accumulators so each arriving x chunk is consumed
            # immediately; V chunks 0/1 here, 2/3 ride window 0's bg queue.
            paccs = [psY.tile([128, 512], F32, tag="y", name=f"pacc_f{fc}")
                     for fc in range(3)]
            for cc in range(CC):
                for fc in range(3):
                    nc.tensor.matmul(
                        paccs[fc],
                        lhsT=wqks[:, cc, fc, :],
                        rhs=xTs[:, cc, 0:512],
                        start=(cc == 0), stop=(cc == CC - 1),
                    )
            for fc in range(3):
                _evac_qk(nc, st, 0, fc, paccs[fc])
            _proj_v_chunk(nc, st, 0)
            _proj_v_chunk(nc, st, 1, acc="y")

            for w in range(TW):
                _attn_window(nc, st, w)

            # epilogue: output projection for the last window, split-phase:
            # the h0/h1 contraction runs over 6 parallel accumulators
            # (borrowing the now-idle psS/psY banks) while the h2 stream's
            # normalize drains, then the h2 matmuls close each group.
            qs = (TW - 1) * 512
            opst = []
            for ec in range(CC):
                pool, tag = [(psY, "y"), (psY, "y"), (psY, "y"),
                             (psS, "s3"), (psS, "s3"), (psA, "acc")][ec]
                opst.append(pool.tile([128, 512], F32, tag=tag,
                                      name=f"opse_{ec}"))
            osb6e = outs.tile([128, CC, 512], F16, tag="osb",
                              name="osb_epi")
            for ec in range(CC):
                for c0 in (0, 256):
                    nc.tensor.matmul(
                        opst[ec][:, c0:c0 + 256],
                        lhsT=wos01[:, ec * 128:(ec + 1) * 128],
                        rhs=ynA[:, qs + c0:qs + c0 + 256],
                        start=(c0 == 0), stop=False, skip_group_check=True,
                    )
                for c0 in (0, 256):
                    nc.tensor.matmul(
                        opst[ec][:, c0:c0 + 256],
                        lhsT=wos2[:, ec * 128:(ec + 1) * 128],
                        rhs=ynB[:, qs + c0:qs + c0 + 256],
                        start=False, stop=True, skip_group_check=True,
                    )
                if ec % 2 == 0:
                    nc.vector.tensor_copy(out=osb6e[:, ec, :], in_=opst[ec])
                else:
                    nc.scalar.copy(out=osb6e[:, ec, :], in_=opst[ec])
                if ec == 3:
                    nc.sync.dma_start(
                        out=outT[0:512, qs:qs + 512].rearrange(
                            "(e p) c -> p e c", e=4),
                        in_=osb6e[:, 0:4, :],
                    )
                elif ec == 5:
                    nc.scalar.dma_start(
                        out=outT[512:768, qs:qs + 512].rearrange(
                            "(e p) c -> p e c", e=2),
                        in_=osb6e[:, 4:6, :],
                    )
    return nc


def _proj_qk_window(nc, st, w):
    """q/k projection for token window w: 3 fc groups of 6 matmuls each,
    bias folded into the DVE evacuation."""
    ts_ = w * 512
    for fc in range(3):
        _proj_qk_group(nc, st, w, fc)
    del ts_


def _proj_qk_group(nc, st, w, fc, acc="acc"):
    ts_ = w * 512
    pool = st["psY"] if acc == "y" else st["psA"]
    ps = pool.tile([128, 512], F32, tag=acc, name=f"ps_f{fc}_t{w}")
    for cc in range(CC):
        nc.tensor.matmul(
            ps,
            lhsT=st["wqks"][:, cc, fc, :],
            rhs=st["xTs"][:, cc, ts_:ts_ + 512],
            start=(cc == 0), stop=(cc == CC - 1),
        )
    _evac_qk(nc, st, w, fc, ps)


def _evac_qk(nc, st, w, fc, ps):
    ts_ = w * 512
    # evacuate with bias add (per-partition scalar).
    # fc0 = [h0q|h1q] -> QQ; fc1 = [h2q|h2k] -> QQ2/KK2; fc2 = [h0k|h1k] -> KK
    if fc == 0:
        nc.vector.tensor_scalar(
            out=st["QQ"][:, ts_:ts_ + 512], in0=ps,
            scalar1=st["bqks"][:, fc:fc + 1], scalar2=None,
            op0=mybir.AluOpType.add)
    elif fc == 2:
        nc.vector.tensor_scalar(
            out=st["KK"][:, ts_:ts_ + 512], in0=ps,
            scalar1=st["bqks"][:, fc:fc + 1], scalar2=None,
            op0=mybir.AluOpType.add)
    else:
        nc.vector.tensor_scalar(
            out=st["QQ2"][:, ts_:ts_ + 512], in0=ps[0:HD, :],
            scalar1=st["bqks"][0:HD, fc:fc + 1], scalar2=None,
            op0=mybir.AluOpType.add)
        nc.vector.tensor_scalar(
            out=st["KK2"][:, ts_:ts_ + 512], in0=ps[HD:128, :],
            scalar1=st["bqks"][HD:128, fc:fc + 1], scalar2=None,
            op0=mybir.AluOpType.add)


def _proj_v_chunk(nc, st, tc_i, acc="acc"):
    """token-major V projection for one 128-token chunk."""
    pool = st["psY"] if acc == "y" else st["psA"]
    psv = pool.tile([128, 512], F32, tag=acc, name=f"psv_{tc_i}")
    for cc in range(CC):
        nc.tensor.matmul(
            psv[:, 0:HPC * HD],
            lhsT=st["xTs"][:, cc, tc_i * 128:(tc_i + 1) * 128],
            rhs=st["wvs"][:, cc, :],
            start=(cc == 0), stop=(cc == CC - 1),
        )
    nc.vector.tensor_copy(
        out=st["vT3"][:, tc_i, :, 0:HD],
        in_=psv[:, 0:HPC * HD].rearrange("p (h d) -> p h d", h=HPC),
    )


def _outproj_group(nc, st, w, ec):
    qs = w * 512
    act_ok = w < TW - 2  # evacs run in window w+1; Act has slack if w+1<=2
    if ec == 0:
        st["osb6"] = st["outs"].tile([128, CC, 512], F16, tag="osb",
                                     name=f"osb_q{w}")
    ops = st["psA"].tile([128, 512], F32, tag="acc", name=f"ops_e{ec}_q{w}")
    nc.tensor.matmul(
        ops,
        lhsT=st["wos01"][:, ec * 128:(ec + 1) * 128],
        rhs=st["ynA"][:, qs:qs + 512],
        start=True, stop=False,
    )
    nc.tensor.matmul(
        ops,
        lhsT=st["wos2"][:, ec * 128:(ec + 1) * 128],
        rhs=st["ynB"][:, qs:qs + 512],
        start=False, stop=True,
    )
    nc.vector.tensor_copy(out=st["osb6"][:, ec, :], in_=ops)
    if ec == CC - 1:
        nc.sync.dma_start(
            out=st["outT"][:, qs:qs + 512].rearrange(
                "(e p) c -> p e c", e=CC),
            in_=st["osb6"],
        )


def _attn_window(nc, st, w):
    """Attention for q-window w across the 3 head streams, with background
    PE work (next-window projection, previous-window outproj) interleaved
    between score/PV rounds."""
    qs = w * 512
    nchunks = 4 * (w + 1)
    # below-diagonal chunks first (descending, so the first PV write is the
    # full column range), diagonal chunks last: the window's own K-side
    # projection (fc2) and V chunks then slide into this window's early
    # rounds instead of crowding the previous one.
    kc_order = list(range(4 * w))[::-1] + list(range(4 * w, nchunks))
    batches = [kc_order[i:i + EXP_BATCH]
               for i in range(0, nchunks, EXP_BATCH)]
    n_diag_batches = 2

    # bg_must: work that must land before this window's diagonal rounds.
    # bg_opt: deferrable work (previous window's normalize phase B and
    # outproj, next window's Q-side projection).
    bg_must = []
    if w == 0:
        for j in (2, 3):
            bg_must.append(lambda j=j: _proj_v_chunk(nc, st, j))
    else:
        bg_must.append(lambda: _proj_qk_group(nc, st, w, 2))
        for j in range(4):
            bg_must.append(lambda j=j: _proj_v_chunk(nc, st, 4 * w + j))
    bg_opt = []
    for s_, w_ in st.pop("pending_norm", []):
        bg_opt.append(lambda s_=s_, w_=w_: _normalize_b(nc, st, s_, w_))
    if w + 1 < TW:
        for fc in (0, 1):
            bg_opt.append(lambda fc=fc: _proj_qk_group(nc, st, w + 1, fc))
    if w >= 1:
        for ec in range(CC):
            bg_opt.append(lambda ec=ec: _outproj_group(nc, st, w - 1, ec))

    n_below = len(batches) - n_diag_batches

    def bg_slot(diag_phase=False):
        if bg_must:
            bg_must.pop(0)()
        elif bg_opt:
            # hold back two opt groups to feed the PE during the
            # Act-bound diagonal rounds
            if diag_phase or len(bg_opt) > 2 or w == 0:
                bg_opt.pop(0)()

    bg = bg_opt  # leftover drain at window end uses the opt queue

    streams = [
        {"h": 0, "qq": st["QQ"], "kk": st["KK"], "rb": 0,
         "yn_ap": lambda q, n: st["ynA"][0:HD, q:q + n]},
        {"h": 1, "qq": st["QQ"], "kk": st["KK"], "rb": HD,
         "yn_ap": lambda q, n: st["ynA"][HD:128, q:q + n]},
        {"h": 2, "qq": st["QQ2"], "kk": st["KK2"], "rb": 0,
         "yn_ap": lambda q, n: st["ynB"][0:HD, q:q + n]},
    ]
    for s in streams:
        s["y"] = st["psY"].tile([128, 512], F32, tag="y",
                                name=f"y_h{s['h']}_q{w}")

    for bi, kcs in enumerate(batches):
        nb = len(kcs)
        diag_phase = bi >= len(batches) - n_diag_batches
        if bi == len(batches) - n_diag_batches:
            while bg_must:
                bg_must.pop(0)()
        # --- scores + exp for all 3 streams; bg slice between h1 and h2 ---
        ebt = {}
        for si, s in enumerate(streams):
            if si == 2:
                bg_slot(diag_phase)
            h = s["h"]
            rb = s["rb"]
            schrau = (SCHRAU and w == TW - 1 and h == 2
                      and all(kc < 4 * w for kc in kcs))
            s_ps = st["psS"].tile([128, EXP_BATCH, 512], F32, tag="s3",
                                  name=f"s_h{h}_q{w}_b{bi}")
            if schrau:
                eti = st["ets"].tile([128, EXP_BATCH, 512], mybir.dt.int16,
                                     tag="et", name=f"e_h{h}_q{w}_b{bi}")
                et = eti.bitcast(mybir.dt.bfloat16)
            else:
                et = st["ets"].tile([128, EXP_BATCH, 512], F16, tag="et",
                                    name=f"e_h{h}_q{w}_b{bi}")
            ebt[h] = et
            js = [max(0, kc - 4 * w) for kc in kcs]
            jw = js if w <= 1 else [min(js)] * nb
            for i in range(nb):
                kc = kcs[i]
                j = jw[i]
                nc.tensor.matmul(
                    s_ps[:, i, 128 * j:512],
                    lhsT=s["kk"][rb:rb + HD, kc * 128:(kc + 1) * 128],
                    rhs=s["qq"][rb:rb + HD, qs + 128 * j:qs + 512],
                    start=True, stop=True,
                )
            if schrau:
                nc.vector.tensor_scalar(
                    out=eti[:, 0:nb, :], in0=s_ps[:, 0:nb, :],
                    scalar1=SCHRAU_A * SCALE, scalar2=SCHRAU_B,
                    op0=mybir.AluOpType.mult, op1=mybir.AluOpType.add)
                continue
            if any(js) and w <= 1:
                # ragged diagonal batch: exp per chunk over exactly the
                # region its score matmul wrote
                for i in range(nb):
                    nc.scalar.activation(
                        out=et[:, i, 128 * js[i]:512],
                        in_=s_ps[:, i, 128 * js[i]:512],
                        func=mybir.ActivationFunctionType.Exp, scale=SCALE,
                    )
            else:
                jm = min(js)
                nc.scalar.activation(
                    out=et[:, 0:nb, 128 * jm:512],
                    in_=s_ps[:, 0:nb, 128 * jm:512],
                    func=mybir.ActivationFunctionType.Exp, scale=SCALE,
                )
            for i in range(nb):
                j = kcs[i] - 4 * w
                if j < 0:
                    continue
                nc.gpsimd.tensor_mul(
                    out=et[:, i, 128 * j:128 * (j + 1)],
                    in0=et[:, i, 128 * j:128 * (j + 1)],
                    in1=st["mask"],
                )
        # --- PV for all 3 streams; bg slice between h1 and h2; on the
        # last round each stream's normalize follows its last PV so the
        # normalize chains overlap the remaining streams' PE work ---
        last_round = bi == len(batches) - 1
        next_last = bi == len(batches) - 2
        for si, s in enumerate(streams):
            if si == 2:
                bg_slot(diag_phase)
            et = ebt[s["h"]]
            for i in range(nb):
                kc = kcs[i]
                j = max(0, kc - 4 * w)
                idx = bi * EXP_BATCH + i
                nc.tensor.matmul(
                    s["y"][0:HD + 1, 128 * j:512],
                    lhsT=st["vT3"][:, kc, s["h"], 0:HD + 1],
                    rhs=et[:, i, 128 * j:512],
                    start=(idx == 0), stop=(idx == nchunks - 1),
                    skip_group_check=True,
                )
            if w == TW - 1:
                # the diagonal chunks only touch ascending column ranges, so
                # sumexp[0:256] is final one batch early: run the normalize
                # chain per column half as it becomes final, overlapping the
                # last batch's score/exp/PV work
                if next_last:
                    _normalize_half(nc, st, s, w, 0)
                elif last_round:
                    _normalize_half(nc, st, s, w, 256)
            elif last_round:
                _normalize_a(nc, st, s, w)
    if w != TW - 1:
        st["pending_norm"] = [(s, w) for s in streams]

    # leftover background groups
    while bg:
        bg.pop(0)()


def _normalize_half(nc, st, s, w, c0):
    """full normalize chain for one 256-wide column half (last window)."""
    qs = w * 512
    h = s["h"]
    y = s["y"]
    if c0 == 0:
        s["sc"] = st["scr"].tile([128, 512], F32, tag="sc",
                                 name=f"sc_h{h}_q{w}")
        s["rbt"] = st["scr"].tile([HD, 512], F32, tag="rbs",
                                  name=f"rb_h{h}_q{w}")
    sc, rb = s["sc"], s["rbt"]
    nc.vector.reciprocal(out=sc[0:1, c0:c0 + 256],
                         in_=y[HD:HD + 1, c0:c0 + 256])
    nc.gpsimd.partition_broadcast(rb[:, c0:c0 + 256], sc[0:1, c0:c0 + 256])
    nc.vector.tensor_mul(
        out=s["yn_ap"](qs + c0, 256),
        in0=y[0:HD, c0:c0 + 256], in1=rb[:, c0:c0 + 256],
    )


def _normalize_a(nc, st, s, w):
    """reciprocal of the sumexp row (column halves on the last window so
    phase B can start earlier; full width otherwise)."""
    h = s["h"]
    y = s["y"]
    sc = st["scr"].tile([128, 512], F32, tag="sc", name=f"sc_h{h}_q{w}")
    s["sc"] = sc
    halves = (0, 256) if w == TW - 1 else (0,)
    wd = 256 if w == TW - 1 else 512
    with nc.allow_low_precision(reason="fp32r == fp32 bits; rounding "
                                "only affects the PE broadcast matmul"):
        for c0 in halves:
            nc.vector.reciprocal(out=sc[0:1, c0:c0 + wd],
                                 in_=y[HD:HD + 1, c0:c0 + wd])


def _normalize_b(nc, st, s, w):
    """broadcast 1/sumexp into the y tile's free partitions 64..127 via a
    K=1 fp32r matmul, then y[0:64] * y[64:128] -> yn (two column halves so
    the output projection can start on the first half early)."""
    qs = w * 512
    h = s["h"]
    y = s["y"]
    sc = s["sc"]
    rb = st["scr"].tile([HD, 512], F32, tag="rbs", name=f"rb_h{h}_q{w}")
    halves = (0, 256) if w == TW - 1 else (0,)
    wd = 256 if w == TW - 1 else 512
    # broadcast 1/sumexp from sc partition 0 to 64 partitions on the GPSIMD
    # engine (SBUF->SBUF; PSUM matmul outputs can't start at partition 64,
    # and the psA bank is contended by background groups)
    for c0 in halves:
        nc.gpsimd.partition_broadcast(rb[:, c0:c0 + wd],
                                      sc[0:1, c0:c0 + wd])
    for c0 in halves:
        nc.vector.tensor_mul(
            out=s["yn_ap"](qs + c0, wd),
            in0=y[0:HD, c0:c0 + wd], in1=rb[:, c0:c0 + wd],
        )


def _prep_core_inputs(c, x, w_qkv, b_qkv, w_out):
    b = c // CPB
    g = c % CPB
    hs = [HPC * g + i for i in range(HPC)]

    qc = [np.arange(h * HD, (h + 1) * HD) for h in hs]
    kc_ = [D + h * HD + np.arange(HD) for h in hs]
    vc = [2 * D + h * HD + np.arange(HD) for h in hs]

    cols = np.concatenate([qc[0], qc[1], qc[2], kc_[2], kc_[0], kc_[1]])
    vcols = np.concatenate(vc)

    xT = np.ascontiguousarray(x[b].T)
    return {
        "xT": np.ascontiguousarray(
            xT.astype(np.float16).reshape(CC, 128, T)),
        "wqk": np.ascontiguousarray(
            w_qkv[:, cols].astype(np.float16).reshape(CC, 128, 3, 128)),
        "bqk": np.ascontiguousarray(
            b_qkv[cols].reshape(3, 128).T.astype(np.float32)),
        "wv": np.ascontiguousarray(
            w_qkv[:, vcols].astype(np.float16).reshape(CC, 128, HPC * HD)),
        "wo01": np.ascontiguousarray(
            w_out[192 * g:192 * g + 128, :].astype(np.float16)),
        "wo2": np.ascontiguousarray(
            w_out[192 * g + 128:192 * g + 192, :].astype(np.float16)),
    }


_NC_CACHE = {}


def get_nc():
    if "nc" not in _NC_CACHE:
        nc = build_bass()
        nc.finalize()
        _NC_CACHE["nc"] = nc
    return _NC_CACHE["nc"]


def kernel(x, w_qkv, b_qkv, w_out, b_out, _run_kwargs=None):
    x = np.asarray(x, dtype=np.float32)
    w_qkv = np.asarray(w_qkv, dtype=np.float32)
    b_qkv = np.asarray(b_qkv, dtype=np.float32)
    w_out = np.asarray(w_out, dtype=np.float32)
    b_out = np.asarray(b_out, dtype=np.float32)

    nc = get_nc()
    in_maps = [_prep_core_inputs(c, x, w_qkv, b_qkv, w_out)
               for c in range(NCORES)]
    kwargs = dict(_run_kwargs or {})
    res = run_bass_kernel_spmd(nc, in_maps, core_ids=list(range(NCORES)),
                               **kwargs)
    if kwargs:
        _NC_CACHE["last_results"] = res

    bv_corr = b_qkv[2 * D:3 * D] @ w_out  # [D]
    out = np.zeros((B, T, D), dtype=np.float32)
    for b in range(B):
        acc = np.zeros((T, D), dtype=np.float32)
        for g in range(CPB):
            acc += np.asarray(res.results[b * CPB + g]["outT"]
                              ).astype(np.float32).T
        out[b] = acc + (b_out + bv_corr)[None, :]
    return out


if __name__ == "__main__":
    nc = build_bass()
    print("built OK")
